# revision 1
# baseline (speedup 1.0000x reference)
"""nn_CNNTransformer Trainium2 kernel — full-input/full-output contract.

Sharding (8 NeuronCores): 2 batch groups x 4 cores.  Within a group each
core computes the QKV convs + attention + partial unify conv for its 2 of
the 8 heads; unify partials (with the residual folded in on exactly one
rank via a one-hot multiplier) are ReduceScattered over the group in 4
tile-quarter chunks, LayerNorm+MLP+LayerNorm run on the core's tile shard,
and chunked AllGathers rebuild the full activation for the next layer.
All matmuls are bf16 with fp32 PSUM accumulation; softmax/LN stats fp32.

Self-contained: only needs the concourse tree at /opt/trn_rl_repo.
"""
import os
import sys
import time

for _p in ("/opt/trn_rl_repo", "/root/.axon_site/_ro/trn_rl_repo"):
    if os.path.isdir(_p) and _p not in sys.path:
        sys.path.insert(0, _p)
        break

import numpy as np
import ml_dtypes

import concourse.bacc as bacc
import concourse.mybir as mybir
import concourse.tile as tile

B, NT, HID, HEADS, L = 2, 16, 64, 8, 2
T = NT * NT            # 256 tiles
P = 256                # pixels per 16x16 tile
HPC = 2                # heads per core
N_CORES = 8
GROUPS = [[0, 1, 2, 3], [4, 5, 6, 7]]
BF16 = mybir.dt.bfloat16
F32 = mybir.dt.float32
NBF = ml_dtypes.bfloat16
AF = mybir.ActivationFunctionType
ALU = mybir.AluOpType

# 3x3 offsets, center first (first matmul covers the full PSUM region).
OFFS = [(1, 1)] + [(dy, dx) for dy in range(3) for dx in range(3)
                   if (dy, dx) != (1, 1)]
OFFIDX = [dy * 3 + dx for dy, dx in OFFS]     # host weight reorder


def _rng(o, n=16):
    return max(o - 1, 0), max(1 - o, 0), n - abs(o - 1)


def build(debug_taps=(), only=None, reps=1):
    def ph_on(p, l=None):
        if only is None:
            return True
        return p in only or (l is not None and f"{p}{l}" in only)
    nc = bacc.Bacc(None, target_bir_lowering=False, debug=False)

    xt = nc.dram_tensor("xt", [3, T, P], BF16, kind="ExternalInput")
    pos_in = nc.dram_tensor("pos_in", [HID, T], F32, kind="ExternalInput")
    semwT = nc.dram_tensor("semwT", [3, HID], BF16, kind="ExternalInput")
    semb = nc.dram_tensor("semb", [HID, 1], F32, kind="ExternalInput")
    qkvwT_in = nc.dram_tensor("qkvwT", [L, HID, 27, 2 * HID], BF16,
                              kind="ExternalInput")
    uwT_in = nc.dram_tensor("uwT", [L, 2 * HID, 9, HID], BF16,
                            kind="ExternalInput")
    m1T_in = nc.dram_tensor("m1T", [L, HID, 4 * HID], BF16,
                            kind="ExternalInput")
    m1b_in = nc.dram_tensor("m1b", [L, 2 * HID, 2], F32,
                            kind="ExternalInput")
    m2T_in = nc.dram_tensor("m2T", [L, 2 * HID, 2, HID], BF16,
                            kind="ExternalInput")
    m2b_in = nc.dram_tensor("m2b", [L, HID, 1], F32, kind="ExternalInput")
    lnw_in = nc.dram_tensor("lnw", [L, 2, HID, P], F32, kind="ExternalInput")
    lnb_in = nc.dram_tensor("lnb", [L, 2, HID, P], F32, kind="ExternalInput")
    outw1T_in = nc.dram_tensor("outw1T", [HID, 9, HID], BF16,
                               kind="ExternalInput")
    outb1_in = nc.dram_tensor("outb1", [HID, 1], F32, kind="ExternalInput")
    outw2T_in = nc.dram_tensor("outw2T", [HID, 4], BF16,
                               kind="ExternalInput")
    outb2_in = nc.dram_tensor("outb2", [4, 1], F32, kind="ExternalInput")
    ident_in = nc.dram_tensor("ident_in", [128, 128], BF16,
                              kind="ExternalInput")
    sel_in = nc.dram_tensor("sel_in", [HID, 4], F32, kind="ExternalInput")
    y_out = nc.dram_tensor("y_out", [4, 4, 16, NT * 16], F32,
                           kind="ExternalOutput")

    taps = {}

    def tap(name, shape, dtype=BF16):
        if name in debug_taps and name not in taps:
            taps[name] = nc.dram_tensor("tap_" + name, shape, dtype,
                                        kind="ExternalOutput")
        return taps.get(name)

    def dump_dram(pool, src_dram, tname, shape, dtype=BF16):
        """Copy a [C, T, P]-like dram tile to a tap via sbuf bounce."""
        for ch in range(8):
            n = shape[1] // 8
            sl = slice(n * ch, n * ch + n)
            stg = pool.tile([shape[0], n, shape[2]], dtype, tag="dumpstg")
            nc.sync.dma_start(stg[:], src_dram[:, sl, :])
            nc.sync.dma_start(taps[tname].ap()[:, sl, :], stg[:])

    with tile.TileContext(nc) as tc:
        with tc.tile_pool(name="dram", bufs=1, space="DRAM") as dram, \
             tc.tile_pool(name="persist", bufs=1) as persist:

            posf_sb = persist.tile([HID, T], F32, tag="posf_sb")
            ident = persist.tile([128, 128], BF16, tag="ident")
            ones64 = persist.tile([HID, HID], F32, tag="ones64")
            sel_sb = persist.tile([HID, 4], F32, tag="sel_sb")
            eps_sb = persist.tile([HID, 1], F32, tag="eps_sb")
            nc.sync.dma_start(posf_sb[:], pos_in.ap())
            nc.sync.dma_start(ident[:], ident_in.ap())
            nc.sync.dma_start(sel_sb[:], sel_in.ap())
            nc.gpsimd.memset(ones64[:], 1.0)
            nc.gpsimd.memset(eps_sb[:], 1e-5)

            for rep in range(reps):
                Qd = dram.tile([128, T, P], BF16, tag=f"Qd{rep}", name="Qd")
                Kd = dram.tile([128, T, P], BF16, tag=f"Kd{rep}", name="Kd")
                Vd = dram.tile([128, T, P], BF16, tag=f"Vd{rep}", name="Vd")
                Od = dram.tile([128, T, P], BF16, tag=f"Od{rep}", name="Od")
                Td = dram.tile([HID, T, P], F32, tag=f"Td{rep}", name="Td")
                # ---------- stage 0: sem 1x1 conv + ReLU + pos -> Td ----------
                with tc.tile_pool(name=f"s0{rep}", bufs=4) as s0, \
                     tc.tile_pool(name=f"s0w{rep}", bufs=1) as s0w, \
                     tc.tile_pool(name=f"s0p{rep}", bufs=6, space="PSUM") as s0p:
                    swt = s0w.tile([3, HID], BF16, tag="swt")
                    sbt = s0w.tile([HID, 1], F32, tag="sbt")
                    if ph_on('S'):
                        nc.sync.dma_start(swt[:], semwT.ap())
                        nc.sync.dma_start(sbt[:], semb.ap())
                    for tp in range(T // 2 if ph_on('S') else 0):
                        ts2 = slice(2 * tp, 2 * tp + 2)
                        xch = s0.tile([3, 2 * P], BF16, tag="xch")
                        nc.sync.dma_start(
                            xch[:], xt.ap()[:, ts2, :].rearrange("c t p -> c (t p)"))
                        ps = s0p.tile([HID, 2 * P], F32, tag="ps")
                        nc.tensor.matmul(ps[:], swt[:], xch[:],
                                         start=True, stop=True)
                        tch = s0.tile([HID, 2, P], F32, tag="tch")
                        nc.scalar.activation(
                            tch.rearrange("c t p -> c (t p)"), ps[:],
                            AF.Relu, bias=sbt[:, 0:1])
                        nc.vector.tensor_add(
                            tch[:], tch[:],
                            posf_sb[:, ts2].unsqueeze(-1)
                            .broadcast_to((HID, 2, P)))
                        nc.sync.dma_start(
                            Td[:, ts2, :].rearrange("c t p -> c (t p)"),
                            tch.rearrange("c t p -> c (t p)"))
                if tap("T0", [HID, T, P]) is not None:
                    with tc.tile_pool(name=f"tp0{rep}", bufs=2) as tp0:
                        dump_dram(tp0, Td, "T0", [HID, T, P])

                # ---------- transformer layers ----------
                for l in range(L):
                    # ---- phase A: QKV convs (stream T chunks from Td) ----
                    pbk_cm = tc.tile_pool(name=f"BK{l}_{rep}", bufs=1)
                    pbk = pbk_cm.__enter__()
                    Ksb = pbk.tile([128, T, P], BF16, tag="Ksb")
                    with tc.tile_pool(name=f"A{l}_{rep}", bufs=6) as pa, \
                         tc.tile_pool(name=f"Aw{l}_{rep}", bufs=1) as paw, \
                         tc.tile_pool(name=f"Ap{l}_{rep}", bufs=6, space="PSUM") as pap:
                        qkvw = paw.tile([HID, 27, 2 * HID], BF16, tag="qkvw")
                        if ph_on('A', l):
                            nc.sync.dma_start(qkvw[:], qkvwT_in.ap()[l])
                        for tp in range(T // 2 if ph_on('A', l) else 0):
                            ts2 = slice(2 * tp, 2 * tp + 2)
                            tchf = pa.tile([HID, 2, 16, 16], F32, tag="tchf")
                            nc.sync.dma_start(
                                tchf.rearrange("c a y x -> c (a y x)"),
                                Td[:, ts2, :].rearrange("c t p -> c (t p)"))
                            tch = pa.tile([HID, 2, 16, 16], BF16, tag="tch")
                            nc.vector.tensor_copy(
                                tch.rearrange("c a y x -> c (a y x)"),
                                tchf.rearrange("c a y x -> c (a y x)"))
                            for conv_i, dstd in enumerate((Qd, Kd, Vd)):
                                ps = pap.tile([128, 2, 16, 16], F32, tag="cps")
                                for d, (dy, dx) in enumerate(OFFS):
                                    liy, loy, cy = _rng(dy)
                                    lix, lox, cx = _rng(dx)
                                    nc.tensor.matmul(
                                        ps[:, 0:2, loy:loy + cy, lox:lox + cx],
                                        qkvw[:, conv_i * 9 + d, :],
                                        tch[:, 0:2, liy:liy + cy, lix:lix + cx],
                                        start=(d == 0), stop=(d == 8),
                                        skip_group_check=True)
                                srcap = ps.rearrange("c a y x -> c (a y x)")
                                if conv_i == 1:
                                    nc.vector.tensor_copy(
                                        Ksb[:, ts2, :].rearrange("c t p -> c (t p)"),
                                        srcap)
                                else:
                                    st = pa.tile([128, 2 * P], BF16, tag="cst")
                                    if conv_i == 2:
                                        nc.scalar.copy(st[:], srcap)
                                    else:
                                        nc.vector.tensor_copy(st[:], srcap)
                                    nc.sync.dma_start(
                                        dstd[:, ts2, :].rearrange("c t p -> c (t p)"),
                                        st[:])

                    # ---- phase B+C: scores, softmax, o ----
                    with tc.tile_pool(name=f"Bs{l}_{rep}", bufs=4) as pbs, \
                         tc.tile_pool(name=f"Bst{l}_{rep}", bufs=1) as pbst:
                        STts = [pbst.tile([128, 2, T], BF16, tag=f"STt{h}",
                                          name=f"STt{h}") for h in range(HPC)]
                        rinvs = {}
                        with tc.tile_pool(name=f"BQ{l}_{rep}", bufs=1) as pbq, \
                             tc.tile_pool(name=f"Bp{l}_{rep}", bufs=2,
                                          space="PSUM") as pbp, \
                             tc.tile_pool(name=f"Bt{l}_{rep}", bufs=2,
                                          space="PSUM") as pbt:
                            if not ph_on('B', l):
                                for jb in range(2):
                                    for h in range(HPC):
                                        rinv = pbs.tile([128, 1], F32,
                                                        tag=f"rinv{jb}{h}",
                                                        bufs=1, name="rinvx")
                                        nc.gpsimd.memset(rinv[:], 1.0)
                                        rinvs[(jb, h)] = rinv
                            for jb in range(2 if ph_on('B', l) else 0):
                                Qsb = pbq.tile([128, 128, P], BF16, tag="Qsb")
                                for ch in range(4):
                                    sl = slice(32 * ch, 32 * ch + 32)
                                    sg = slice(128 * jb + 32 * ch,
                                               128 * jb + 32 * ch + 32)
                                    nc.sync.dma_start(Qsb[:, sl, :], Qd[:, sg, :])
                                for h in range(HPC):
                                    hs = slice(64 * h, 64 * h + 64)
                                    sps = pbp.tile([128, T], F32, tag="sps")
                                    for p in range(P):
                                        nc.tensor.matmul(
                                            sps[:], Qsb[hs, :, p], Ksb[hs, :, p],
                                            start=(p == 0), stop=(p == P - 1))
                                    nmax = pbs.tile([128, 1], F32, tag="nmax")
                                    nc.vector.reduce_max(
                                        nmax[:], sps[:],
                                        axis=mybir.AxisListType.X, negate=True)
                                    E = pbs.tile([128, T], BF16, tag="E")
                                    esum = pbs.tile([128, 1], F32, tag="esum")
                                    nc.scalar.activation(E[:], sps[:], AF.Exp,
                                                         bias=nmax[:, 0:1],
                                                         accum_out=esum[:])
                                    rinv = pbs.tile([128, 1], F32,
                                                    tag=f"rinv{jb}{h}", bufs=1)
                                    nc.vector.reciprocal(rinv[:], esum[:])
                                    rinvs[(jb, h)] = rinv
                                    for qb in range(2):
                                        tps = pbt.tile([128, 128], BF16,
                                                       tag="tps")
                                        nc.tensor.matmul(
                                            tps[:],
                                            E[:, 128 * qb:128 * qb + 128],
                                            ident[:], is_transpose=True)
                                        nc.scalar.copy(
                                            STts[h][:, qb,
                                                    128 * jb:128 * jb + 128],
                                            tps[:])
                                    if l == 0 and jb == 0 and h == 0 and \
                                            tap("E00", [128, 1, T]) is not None:
                                        nc.sync.dma_start(
                                            taps["E00"].ap()
                                            .rearrange("a b c -> a (b c)"), E[:])

                        # phase C (K pool still open; Q closed)
                        with tc.tile_pool(name=f"C{l}_{rep}", bufs=1) as pc, \
                             tc.tile_pool(name=f"Cs{l}_{rep}", bufs=4) as pcs, \
                             tc.tile_pool(name=f"Cp{l}_{rep}", bufs=4,
                                          space="PSUM") as pcp:
                            for h in range(HPC if ph_on('C', l) else 0):
                                vts = []
                                for qb in range(2):
                                    Vt = pc.tile([128, HID, P], BF16,
                                                 tag=f"Vt{qb}")
                                    nc.sync.dma_start(
                                        Vt[:],
                                        Vd[64 * h:64 * h + 64,
                                           128 * qb:128 * qb + 128, :]
                                        .rearrange("c q p -> q c p"))
                                    vts.append(Vt)
                                for jb in range(2):
                                    rinv = rinvs[(jb, h)]
                                    for ch in range(HID // 2):
                                        ops = pcp.tile([128, 2 * P], F32,
                                                       tag="ops")
                                        for qb in range(2):
                                            nc.tensor.matmul(
                                                ops[:],
                                                STts[h][:, qb,
                                                        128 * jb:128 * jb + 128],
                                                vts[qb][:, 2 * ch:2 * ch + 2, :]
                                                .rearrange("q c p -> q (c p)"),
                                                start=(qb == 0), stop=(qb == 1))
                                        ost = pcs.tile([128, 2 * P], BF16,
                                                       tag="ost")
                                        nc.vector.tensor_scalar_mul(
                                            ost[:], ops[:], rinv[:, 0:1])
                                        nc.sync.dma_start(
                                            Od[64 * h + 2 * ch:
                                               64 * h + 2 * ch + 2,
                                               128 * jb:128 * jb + 128, :]
                                            .rearrange("c j p -> j c p"),
                                            ost.rearrange("j (c p) -> j c p",
                                                          c=2))

                    pbk_cm.__exit__(None, None, None)
                    # ---- phase D: unify conv + residual fold + RS ----
                    Ud = [dram.tile([4, HID, 16, P], F32, tag=f"Ud{q}_{rep}",
                                     name=f"Ud{q}") for q in range(4)]
                    RSo = [dram.tile([HID, 16, P], F32, tag=f"RSo{q}_{rep}",
                                      name=f"RSo{q}") for q in range(4)]
                    with tc.tile_pool(name=f"D{l}_{rep}", bufs=4) as pd, \
                         tc.tile_pool(name=f"Dw{l}_{rep}", bufs=1) as pdw, \
                         tc.tile_pool(name=f"Dp{l}_{rep}", bufs=4, space="PSUM") as pdp:
                        uw = pdw.tile([2 * HID, 9, HID], BF16, tag="uw")
                        if ph_on('D', l):
                            nc.sync.dma_start(uw[:], uwT_in.ap()[l])
                        for tp in range(T // 2 if ph_on('D', l) else 0):
                            g0 = 2 * tp
                            q, r = g0 // 64, (g0 % 64) // 16
                            och = pd.tile([128, 2, 16, 16], BF16, tag="och")
                            nc.sync.dma_start(
                                och.rearrange("c a y x -> c (a y x)"),
                                Od[:, g0:g0 + 2, :].rearrange("c t p -> c (t p)"))
                            ups = pdp.tile([HID, 2, 16, 16], F32, tag="ups")
                            for d, (dy, dx) in enumerate(OFFS):
                                liy, loy, cy = _rng(dy)
                                lix, lox, cx = _rng(dx)
                                nc.tensor.matmul(
                                    ups[:, 0:2, loy:loy + cy, lox:lox + cx],
                                    uw[:, d, :],
                                    och[:, 0:2, liy:liy + cy, lix:lix + cx],
                                    start=(d == 0), stop=(d == 8),
                                    skip_group_check=True)
                            tsl = pd.tile([HID, 2 * P], F32, tag="tsl")
                            nc.sync.dma_start(
                                tsl[:],
                                Td[:, g0:g0 + 2, :].rearrange("c t p -> c (t p)"))
                            ust = pd.tile([HID, 2 * P], F32, tag="ust")
                            nc.vector.scalar_tensor_tensor(
                                ust[:], tsl[:], sel_sb[:, r:r + 1],
                                ups.rearrange("c a y x -> c (a y x)"),
                                op0=ALU.mult, op1=ALU.add)
                            s = g0 % 16
                            nc.sync.dma_start(
                                Ud[q][r, :, s:s + 2, :]
                                .rearrange("c t p -> c (t p)"), ust[:])
                            if g0 % 64 == 62 and ph_on('R', l):
                                nc.gpsimd.collective_compute(
                                    "ReduceScatter", ALU.add,
                                    replica_groups=GROUPS,
                                    ins=[Ud[q].opt()], outs=[RSo[q].opt()])

                    # ---- phase E: LN1 + MLP + LN2 per quarter + AG ----
                    AGi = [dram.tile([HID, 16, P], F32, tag=f"AGi{q}_{rep}",
                                      name=f"AGi{q}") for q in range(4)]
                    AGo = [dram.tile([4, HID, 16, P], F32, tag=f"AGo{q}_{rep}",
                                      name=f"AGo{q}") for q in range(4)]
                    with tc.tile_pool(name=f"E{l}_{rep}", bufs=1) as pe, \
                         tc.tile_pool(name=f"Es{l}_{rep}", bufs=2) as pes, \
                         tc.tile_pool(name=f"Ew{l}_{rep}", bufs=1) as pew, \
                         tc.tile_pool(name=f"Ep{l}_{rep}", bufs=2, space="PSUM") as pep:
                        m1w = pew.tile([HID, 4 * HID], BF16, tag="m1w")
                        m1bs = pew.tile([2 * HID, 2], F32, tag="m1bs")
                        m2w = pew.tile([2 * HID, 2, HID], BF16, tag="m2w")
                        m2bs = pew.tile([HID, 1], F32, tag="m2bs")
                        ln_w = pew.tile([HID, 2, P], F32, tag="ln_w")
                        ln_b = pew.tile([HID, 2, P], F32, tag="ln_b")
                        if ph_on('E', l):
                            nc.sync.dma_start(m1w[:], m1T_in.ap()[l])
                            nc.sync.dma_start(m1bs[:], m1b_in.ap()[l])
                            nc.sync.dma_start(m2w[:], m2T_in.ap()[l])
                            nc.sync.dma_start(m2bs[:], m2b_in.ap()[l])
                        if ph_on('E', l):
                            nc.sync.dma_start(ln_w[:], lnw_in.ap()[l].rearrange("w c p -> c w p"))
                            nc.sync.dma_start(ln_b[:], lnb_in.ap()[l].rearrange("w c p -> c w p"))

                        def layernorm(xf32, wi, out_bf, out_f32=None):
                            stats = pes.tile([HID, 32], F32, tag="stats")
                            nc.vector.reduce_sum(stats[:, 0:16], xf32[:],
                                                 axis=mybir.AxisListType.X)
                            xsq = pe.tile([HID, 16, P], F32, tag="scratch_f")
                            nc.vector.tensor_mul(xsq[:], xf32[:], xf32[:])
                            nc.vector.reduce_sum(stats[:, 16:32], xsq[:],
                                                 axis=mybir.AxisListType.X)
                            sp = pep.tile([HID, 32], F32, tag="sp")
                            nc.tensor.matmul(sp[:], ones64[:], stats[:],
                                             start=True, stop=True)
                            mu = pes.tile([HID, 48], F32, tag="mu")
                            nc.scalar.mul(mu[:, 0:16], sp[:, 0:16], 1.0 / 16384)
                            nc.scalar.mul(mu[:, 16:32], sp[:, 16:32], 1.0 / 16384)
                            nc.scalar.square(mu[:, 32:48], mu[:, 0:16])
                            var = pes.tile([HID, 32], F32, tag="var")
                            nc.vector.tensor_sub(var[:, 0:16], mu[:, 16:32],
                                                 mu[:, 32:48])
                            nc.scalar.activation(var[:, 16:32], var[:, 0:16],
                                                 AF.Sqrt,
                                                 bias=eps_sb[:, 0:1])
                            rstd = pes.tile([HID, 16], F32, tag="rstd")
                            nc.vector.reciprocal(rstd[:], var[:, 16:32])
                            mu_bc = mu[:, 0:16].unsqueeze(-1) \
                                .broadcast_to((HID, 16, P))
                            rs_bc = rstd[:, 0:16].unsqueeze(-1) \
                                .broadcast_to((HID, 16, P))
                            xn = pe.tile([HID, 16, P], F32, tag="scratch_f")
                            nc.vector.tensor_sub(xn[:], xf32[:], mu_bc)
                            nc.vector.tensor_mul(xn[:], xn[:], rs_bc)
                            w_bc = ln_w[:, wi, :].unsqueeze(1).broadcast_to(
                                (HID, 16, P))
                            b_bc = ln_b[:, wi, :].unsqueeze(1).broadcast_to(
                                (HID, 16, P))
                            nc.vector.tensor_mul(xn[:], xn[:], w_bc)
                            tgt = out_f32 if out_f32 is not None else xn
                            nc.vector.tensor_add(tgt[:], xn[:], b_bc)
                            nc.vector.tensor_copy(out_bf[:], tgt[:])

                        for q in range(4 if ph_on('E', l) else 0):
                            x_id = pe.tile([HID, 16, P], F32, tag="x_id")
                            nc.sync.dma_start(x_id[:], RSo[q][:])
                            idn_f = pe.tile([HID, 16, P], F32, tag="idn_f")
                            idn_b = pe.tile([HID, 16, P], BF16, tag="idn_b")
                            layernorm(x_id, 0, idn_b, idn_f)
                            m1sb = pe.tile([128, 2, 16, P], BF16, tag="m1sb")
                            for mb in range(2):
                                for ch in range(8):
                                    c2 = slice(2 * ch, 2 * ch + 2)
                                    mp = pep.tile([128, 2 * P], F32, tag="mp")
                                    nc.tensor.matmul(
                                        mp[:], m1w[:, 128 * mb:128 * mb + 128],
                                        idn_b[:, c2, :]
                                        .rearrange("c t p -> c (t p)"),
                                        start=True, stop=True)
                                    nc.scalar.activation(
                                        m1sb[:, mb, c2, :]
                                        .rearrange("c t p -> c (t p)"),
                                        mp[:], AF.Relu,
                                        bias=m1bs[:, mb:mb + 1])
                            x2f = pe.tile([HID, 16, P], F32, tag="x2f")
                            for ch in range(8):
                                c2 = slice(2 * ch, 2 * ch + 2)
                                mp2 = pep.tile([HID, 2 * P], F32, tag="mp2")
                                for mb in range(2):
                                    nc.tensor.matmul(
                                        mp2[:], m2w[:, mb, :],
                                        m1sb[:, mb, c2, :]
                                        .rearrange("c t p -> c (t p)"),
                                        start=(mb == 0), stop=(mb == 1))
                                nc.vector.scalar_tensor_tensor(
                                    x2f[:, c2, :].rearrange("c t p -> c (t p)"),
                                    mp2[:], m2bs[:, 0:1],
                                    idn_f[:, c2, :].rearrange("c t p -> c (t p)"),
                                    op0=ALU.add, op1=ALU.add)
                            y2f = pe.tile([HID, 16, P], F32, tag="y2f")
                            layernorm(x2f, 1, y2f)
                            nc.sync.dma_start(AGi[q][:], y2f[:])
                            if ph_on('R', l):
                                nc.gpsimd.collective_compute(
                                    "AllGather", ALU.bypass,
                                    replica_groups=GROUPS,
                                    ins=[AGi[q].opt()], outs=[AGo[q].opt()])
                    # write gathered T back to Td (DRAM->DRAM)
                    for q in range(4 if ph_on('R', l) else 0):
                        nc.sync.dma_start(
                            Td[:, 64 * q:64 * q + 64, :]
                            .rearrange("c (r s) p -> c r s p", r=4),
                            AGo[q].rearrange("r c s p -> c r s p"))
                    if l == 0 and tap("T1", [HID, T, P]) is not None:
                        with tc.tile_pool(name=f"tp1{rep}", bufs=2) as tp1:
                            dump_dram(tp1, Td, "T1", [HID, T, P])

                # ---------- output head (rank-symmetric tile-row bands) --------
                with tc.tile_pool(name=f"H{rep}", bufs=2) as ph, \
                     tc.tile_pool(name=f"Hw{rep}", bufs=1) as phw, \
                     tc.tile_pool(name=f"Hp{rep}", bufs=4, space="PSUM") as php:
                    ow1 = phw.tile([HID, 9, HID], BF16, tag="ow1")
                    ob1 = phw.tile([HID, 1], F32, tag="ob1")
                    ow2 = phw.tile([HID, 4], BF16, tag="ow2")
                    ob2 = phw.tile([4, 1], F32, tag="ob2")
                    if ph_on('H'):
                        nc.sync.dma_start(ow1[:], outw1T_in.ap())
                        nc.sync.dma_start(ob1[:], outb1_in.ap())
                        nc.sync.dma_start(ow2[:], outw2T_in.ap())
                        nc.sync.dma_start(ob2[:], outb2_in.ap())
                    for q in range(4 if ph_on('H') else 0):
                        img = ph.tile([HID, 18, 16, 16], F32, tag="img")
                        nc.gpsimd.memset(
                            img.rearrange("c a t x -> c (a t x)"), 0.0)
                        for r in range(4):
                            m = 4 * q + r
                            sel = sel_sb[:, r:r + 1]
                            def edgerow(mm, y, tg):
                                stg = ph.tile([HID, 16 * 16], F32, tag=tg,
                                              name=f"stg_{tg}")
                                nc.sync.dma_start(
                                    stg.rearrange("c (t x) -> c t x", t=16),
                                    Td[:, 16 * mm:16 * mm + 16, :]
                                    .rearrange("c t (y x) -> c t y x", y=16)
                                    [:, :, y, :])
                                return stg
                            segs = []
                            if m >= 1:
                                pv = edgerow(m - 1, 15, "pstg")
                                segs.append((img[:, 0, :, :],
                                             pv.rearrange("c (t x) -> c t x",
                                                          t=16)))
                            mn = ph.tile([HID, 16, 16, 16], F32, tag="mstg",
                                         name="stg_m")
                            nc.sync.dma_start(
                                mn.rearrange("c t y x -> c (t y x)"),
                                Td[:, 16 * m:16 * m + 16, :]
                                .rearrange("c t p -> c (t p)"))
                            for y in range(16):
                                segs.append((img[:, 1 + y, :, :],
                                             mn[:, :, y, :]))
                            if m <= 14:
                                nx = edgerow(m + 1, 0, "nstg")
                                segs.append((img[:, 17, :, :],
                                             nx.rearrange("c (t x) -> c t x",
                                                          t=16)))
                            for dst, srcap in segs:
                                nc.vector.scalar_tensor_tensor(
                                    dst, srcap, sel, dst,
                                    op0=ALU.mult, op1=ALU.add)
                        imgb = ph.tile([HID, 18, 16, 16], BF16, tag="imgb")
                        nc.vector.tensor_copy(
                            imgb.rearrange("c a t x -> c (a t x)"),
                            img.rearrange("c a t x -> c (a t x)"))
                        imgf = imgb.rearrange("c a t x -> c a (t x)")
                        for ch in range(8):
                            oc1 = php.tile([HID, 2, 256], F32, tag="oc1")
                            for d, (dy, dx) in enumerate(OFFS):
                                lix, lox, cx = _rng(dx, 256)
                                nc.tensor.matmul(
                                    oc1[:, 0:2, lox:lox + cx],
                                    ow1[:, d, :],
                                    imgf[:, 2 * ch + dy:2 * ch + dy + 2,
                                         lix:lix + cx],
                                    start=(d == 0), stop=(d == 8),
                                    skip_group_check=True)
                            o1 = ph.tile([HID, 2 * 256], BF16, tag="o1")
                            nc.scalar.activation(
                                o1[:], oc1.rearrange("c a x -> c (a x)"),
                                AF.Relu, bias=ob1[:, 0:1])
                            p2 = php.tile([4, 2 * 256], F32, tag="p2")
                            nc.tensor.matmul(p2[:], ow2[:], o1[:],
                                             start=True, stop=True)
                            ysb = ph.tile([4, 2 * 256], F32, tag="ysb")
                            nc.vector.tensor_scalar_add(ysb[:], p2[:],
                                                        ob2[:, 0:1])
                            nc.sync.dma_start(
                                y_out.ap()[:, q, 2 * ch:2 * ch + 2, :]
                                .rearrange("c t x -> c (t x)"), ysb[:])
    nc.finalize()
    return nc, taps


# ======================= host side =======================

def _prep_core_inputs(core, inputs):
    r = core % 4
    b = core // 4
    f32 = np.float32
    bf = lambda a: np.ascontiguousarray(np.asarray(a, f32)).astype(NBF)
    x = np.asarray(inputs["x"], f32)
    xtc = x[b].reshape(3, 16, 16, 16, 16).transpose(0, 1, 3, 2, 4) \
        .reshape(3, T, P)
    pos = np.asarray(inputs["pos"], f32).reshape(T, HID).T.copy()
    hsl = slice(2 * r * HID, 2 * r * HID + 2 * HID)

    qkv = np.empty((L, HID, 27, 2 * HID), f32)
    for l in range(L):
        for i, (nm, sc) in enumerate((("qw", 0.25), ("kw", 0.25),
                                      ("vw", 1.0))):
            w = np.asarray(inputs[nm], f32)[l, hsl] * sc  # [128,64,3,3]
            wt = w.transpose(1, 2, 3, 0).reshape(HID, 9, 2 * HID)
            qkv[l, :, 9 * i:9 * i + 9, :] = wt[:, OFFIDX, :]
    uwp = np.empty((L, 2 * HID, 9, HID), f32)
    for l in range(L):
        w = np.asarray(inputs["uw"], f32)[l][:, hsl]      # [64,128,3,3]
        wt = w.transpose(1, 2, 3, 0).reshape(2 * HID, 9, HID)
        uwp[l] = wt[:, OFFIDX, :]
    m1 = np.asarray(inputs["mlp_w1"], f32)[:, :, :, 0, 0]  # [L,256,64]
    m1Tp = m1.transpose(0, 2, 1).copy()                    # [L,64,256]
    m1bp = np.asarray(inputs["mlp_b1"], f32).reshape(L, 2, 2 * HID) \
        .transpose(0, 2, 1).copy()                         # [L,128,2]
    m2 = np.asarray(inputs["mlp_w2"], f32)[:, :, :, 0, 0]  # [L,64,256]
    m2Tp = m2.transpose(0, 2, 1).reshape(L, 2, 2 * HID, HID).transpose(0, 2, 1, 3).copy()
    m2bp = np.asarray(inputs["mlp_b2"], f32).reshape(L, HID, 1)
    lnwp = np.stack([np.asarray(inputs["ln1_w"], f32).reshape(L, HID, P),
                     np.asarray(inputs["ln2_w"], f32).reshape(L, HID, P)], 1)
    lnbp = np.stack([np.asarray(inputs["ln1_b"], f32).reshape(L, HID, P),
                     np.asarray(inputs["ln2_b"], f32).reshape(L, HID, P)], 1)
    ow1 = np.asarray(inputs["out_w1"], f32)                # [64,64,3,3]
    ow1T = ow1.transpose(1, 2, 3, 0).reshape(HID, 9, HID)[:, OFFIDX, :]
    ow2 = np.asarray(inputs["out_w2"], f32)[:, :, 0, 0]    # [3,64]
    ow2T = np.zeros((HID, 4), f32)
    ow2T[:, :3] = ow2.T
    ob2 = np.zeros((4, 1), f32)
    ob2[:3, 0] = np.asarray(inputs["out_b2"], f32)
    sel = np.zeros((HID, 4), f32)
    sel[:, r] = 1.0

    return {
        "xt": bf(xtc), "pos_in": pos.astype(np.float32),
        "semwT": bf(np.asarray(inputs["sem_w"], f32)[:, :, 0, 0].T),
        "semb": np.asarray(inputs["sem_b"], f32).reshape(HID, 1).copy(),
        "qkvwT": bf(qkv), "uwT": bf(uwp), "m1T": bf(m1Tp), "m1b": m1bp,
        "m2T": bf(m2Tp), "m2b": m2bp, "lnw": lnwp, "lnb": lnbp,
        "outw1T": bf(ow1T),
        "outb1": np.asarray(inputs["out_b1"], f32).reshape(HID, 1).copy(),
        "outw2T": bf(ow2T), "outb2": ob2,
        "ident_in": np.eye(128, dtype=NBF), "sel_in": sel,
    }


def assemble_output(results):
    img = np.zeros((B, 3, 256, 256), np.float32)
    for c in range(N_CORES):
        b, r = c // 4, c % 4
        y = np.asarray(results[c]["y_out"], np.float32)  # [4,4,16,256]
        for q in range(4):
            rb = 16 * (4 * q + r)
            img[b, :, rb:rb + 16, :] = y[:3, q]
    return img


_CACHE = {}


def get_built(debug_taps=()):
    key = tuple(sorted(debug_taps))
    if key not in _CACHE:
        t0 = time.time()
        nc, taps = build(debug_taps)
        _CACHE[key] = (nc, taps)
        print(f"[kernel] build {time.time() - t0:.1f}s", file=sys.stderr)
    return _CACHE[key]


def run_cores(inputs, debug_taps=()):
    from concourse import bass2jax
    nc, taps = get_built(debug_taps)
    in_maps = [_prep_core_inputs(c, inputs) for c in range(N_CORES)]
    t0 = time.time()
    results = bass2jax.run_bass_via_pjrt(nc, in_maps, n_cores=N_CORES)
    print(f"[kernel] run {time.time() - t0:.1f}s", file=sys.stderr)
    return results


def kernel(**inputs):
    results = run_cores(inputs)
    return assemble_output(results)



# revision 7
# speedup vs baseline: 1.3774x; 1.3774x over previous
"""nn_CNNTransformer Trainium2 kernel — full-input/full-output contract.

Sharding (8 NeuronCores): 2 batch groups x 4 cores.  Within a group each
core computes the QKV convs + attention + partial unify conv for its 2 of
the 8 heads; unify partials (with the residual folded in on exactly one
rank via a one-hot multiplier) are ReduceScattered (bf16) over the group
in 4 tile-quarter chunks, LayerNorm+MLP+LayerNorm run on the core's tile
shard (two quarters at a time across the 128 partitions), and chunked
AllGathers rebuild the full activation for the next layer.

v2: all K=64 contractions are row-tiled across the two 64-partition
halves of the PE array (two concurrent matmuls in different row groups,
two PSUM banks, DVE add at eviction) and 64-col outputs are col-tiled —
this keeps the PE at the warm 2.4 GHz clock and roughly doubles matmul
throughput.  Activations live in DRAM as bf16.

Self-contained: only needs the concourse tree at /opt/trn_rl_repo.
"""
import os
import sys
import time

for _p in ("/opt/trn_rl_repo", "/root/.axon_site/_ro/trn_rl_repo"):
    if os.path.isdir(_p) and _p not in sys.path:
        sys.path.insert(0, _p)
        break

import numpy as np
import ml_dtypes

import concourse.bacc as bacc
import concourse.mybir as mybir
import concourse.tile as tile

B, NT, HID, HEADS, L = 2, 16, 64, 8, 2
T = NT * NT            # 256 tiles
P = 256                # pixels per 16x16 tile
N_CORES = 8
GROUPS = [[0, 1, 2, 3], [4, 5, 6, 7]]
BF16 = mybir.dt.bfloat16
F32 = mybir.dt.float32
NBF = ml_dtypes.bfloat16
AF = mybir.ActivationFunctionType
ALU = mybir.AluOpType

# 3x3 offsets, center first (full-region matmul opens each PSUM bank).
OFFS = [(1, 1)] + [(dy, dx) for dy in range(3) for dx in range(3)
                   if (dy, dx) != (1, 1)]
# bank0 gets half-center + OFFS[1,3,5,7]; bank1 half-center + OFFS[2,4,6,8]
OFFA = [OFFS[i] for i in (0, 1, 3, 5, 7)]
OFFB = [OFFS[i] for i in (0, 2, 4, 6, 8)]


def _rng(o, n=16):
    return max(o - 1, 0), max(1 - o, 0), n - abs(o - 1)


def build(debug_taps=(), only=None, reps=1):
    def ph_on(p, l=None):
        if only is None:
            return True
        return p in only or (l is not None and f"{p}{l}" in only)
    nc = bacc.Bacc(None, target_bir_lowering=False, debug=False)

    xt = nc.dram_tensor("xt", [3, T, P], BF16, kind="ExternalInput")
    pos_in = nc.dram_tensor("pos_in", [HID, T], F32, kind="ExternalInput")
    semwT = nc.dram_tensor("semwT", [3, HID], BF16, kind="ExternalInput")
    semb = nc.dram_tensor("semb", [HID, 1], F32, kind="ExternalInput")
    qkvw2_in = nc.dram_tensor("qkvw2", [L, 128, 3, 5, 128], BF16,
                              kind="ExternalInput")
    uwT_in = nc.dram_tensor("uwT", [L, 2 * HID, 9, HID], BF16,
                            kind="ExternalInput")
    m1w2_in = nc.dram_tensor("m1w2", [L, 128, 2, 128], BF16,
                             kind="ExternalInput")
    m1b_in = nc.dram_tensor("m1b", [L, 2 * HID, 2], F32,
                            kind="ExternalInput")
    m2T_in = nc.dram_tensor("m2T", [L, 2 * HID, 2, HID], BF16,
                            kind="ExternalInput")
    m2b2_in = nc.dram_tensor("m2b2", [L, 128, 1], F32, kind="ExternalInput")
    lnw_in = nc.dram_tensor("lnw", [L, 2, HID, P], F32, kind="ExternalInput")
    lnb_in = nc.dram_tensor("lnb", [L, 2, HID, P], F32, kind="ExternalInput")
    outw1b2_in = nc.dram_tensor("outw1b2", [128, 5, HID], BF16,
                                kind="ExternalInput")
    outb1_in = nc.dram_tensor("outb1", [HID, 1], F32, kind="ExternalInput")
    outw2T_in = nc.dram_tensor("outw2T", [HID, 4], BF16,
                               kind="ExternalInput")
    outb2_in = nc.dram_tensor("outb2", [4, 1], F32, kind="ExternalInput")
    ident_in = nc.dram_tensor("ident_in", [128, 128], BF16,
                              kind="ExternalInput")
    sel_in = nc.dram_tensor("sel_in", [128, 4], F32, kind="ExternalInput")
    onesblk_in = nc.dram_tensor("onesblk_in", [128, 128], F32,
                                kind="ExternalInput")
    y_out = nc.dram_tensor("y_out", [4, 4, 16, NT * 16], F32,
                           kind="ExternalOutput")

    taps = {}

    def tap(name, shape, dtype=BF16):
        if name in debug_taps and name not in taps:
            taps[name] = nc.dram_tensor("tap_" + name, shape, dtype,
                                        kind="ExternalOutput")
        return taps.get(name)

    with tile.TileContext(nc) as tc:
        with tc.tile_pool(name="dram", bufs=1, space="DRAM") as dram, \
             tc.tile_pool(name="persist", bufs=1) as persist:

            posf_sb = persist.tile([HID, T], F32, tag="posf_sb")
            ident = persist.tile([128, 128], BF16, tag="ident")
            onesblk = persist.tile([128, 128], F32, tag="onesblk")
            sel_sb = persist.tile([128, 4], F32, tag="sel_sb")
            eps_sb = persist.tile([128, 1], F32, tag="eps_sb")
            nc.sync.dma_start(posf_sb[:], pos_in.ap())
            nc.sync.dma_start(ident[:], ident_in.ap())
            nc.sync.dma_start(onesblk[:], onesblk_in.ap())
            nc.sync.dma_start(sel_sb[:], sel_in.ap())
            nc.gpsimd.memset(eps_sb[:], 1e-5)

            for rep in range(reps):
                Qd = dram.tile([128, T, P], BF16, tag=f"Qd{rep}", name="Qd")
                Vd = dram.tile([128, T, P], BF16, tag=f"Vd{rep}", name="Vd")
                Od = dram.tile([128, T, P], BF16, tag=f"Od{rep}", name="Od")
                Td = dram.tile([HID, T, P], BF16, tag=f"Td{rep}", name="Td")
                # ---------- stage 0: sem 1x1 conv + ReLU + pos -> Td -------
                with tc.tile_pool(name=f"s0{rep}", bufs=3) as s0, \
                     tc.tile_pool(name=f"s0w{rep}", bufs=1) as s0w, \
                     tc.tile_pool(name=f"s0p{rep}", bufs=2, space="PSUM") as s0p:
                    swt = s0w.tile([128, HID], BF16, tag="swt")
                    sbt = s0w.tile([HID, 1], F32, tag="sbt")
                    if ph_on('S'):
                        nc.sync.dma_start(swt[0:3, :], semwT.ap())
                        nc.sync.dma_start(swt[64:67, :], semwT.ap())
                        nc.sync.dma_start(sbt[:], semb.ap())
                    for i in range(T // 4 if ph_on('S') else 0):
                        g0 = 4 * i
                        xs = s0.tile([128, 2, P], BF16, tag="xs")
                        nc.sync.dma_start(
                            xs[0:3, :, :].rearrange("c t p -> c (t p)"),
                            xt.ap()[:, g0:g0 + 2, :]
                            .rearrange("c t p -> c (t p)"))
                        nc.sync.dma_start(
                            xs[64:67, :, :].rearrange("c t p -> c (t p)"),
                            xt.ap()[:, g0 + 2:g0 + 4, :]
                            .rearrange("c t p -> c (t p)"))
                        psA = s0p.tile([HID, 2 * P], F32, tag="psA")
                        psB = s0p.tile([HID, 2 * P], F32, tag="psB")
                        nc.tensor.matmul(
                            psA[:], swt[0:3, :],
                            xs[0:3, :, :].rearrange("c t p -> c (t p)"),
                            start=True, stop=True)
                        nc.tensor.matmul(
                            psB[:], swt[64:67, :],
                            xs[64:67, :, :].rearrange("c t p -> c (t p)"),
                            start=True, stop=True)
                        for half, ps in ((0, psA), (1, psB)):
                            g = g0 + 2 * half
                            tch = s0.tile([HID, 2, P], F32, tag=f"tch{half}")
                            nc.scalar.activation(
                                tch.rearrange("c t p -> c (t p)"), ps[:],
                                AF.Relu, bias=sbt[:, 0:1])
                            ob = s0.tile([HID, 2, P], BF16, tag=f"ob{half}")
                            eng = nc.vector if half == 0 else nc.gpsimd
                            eng.tensor_add(
                                ob[:], tch[:],
                                posf_sb[:, g:g + 2].unsqueeze(-1)
                                .broadcast_to((HID, 2, P)))
                            nc.sync.dma_start(
                                Td[:, g:g + 2, :]
                                .rearrange("c t p -> c (t p)"),
                                ob.rearrange("c t p -> c (t p)"))

                # ---------- transformer layers ----------
                for l in range(L):
                    # ---- phase A: QKV convs, row-tiled halves ----
                    pbst_cm = tc.tile_pool(name=f"Bst{l}_{rep}", bufs=1)
                    pbst = pbst_cm.__enter__()
                    STts = [pbst.tile([128, 2, T], BF16, tag=f"STt{h}",
                                      name=f"STt{h}") for h in range(2)]
                    pbk_cm = tc.tile_pool(name=f"BK{l}_{rep}", bufs=1)
                    pbk = pbk_cm.__enter__()
                    Ksb = pbk.tile([128, T, P], BF16, tag="Ksb")
                    with tc.tile_pool(name=f"A{l}_{rep}", bufs=3) as pa, \
                         tc.tile_pool(name=f"Aw{l}_{rep}", bufs=1) as paw, \
                         tc.tile_pool(name=f"Ap{l}_{rep}", bufs=1,
                                      space="PSUM") as pap, \
                         tc.tile_pool(name=f"As{l}_{rep}", bufs=3) as pas:
                        qkvw = paw.tile([128, 3, 5, 128], BF16, tag="qkvw")
                        if ph_on('A', l):
                            nc.sync.dma_start(qkvw[:], qkvw2_in.ap()[l])
                        for tp in range(T // 2 if ph_on('A', l) else 0):
                            ts2 = slice(2 * tp, 2 * tp + 2)
                            S = pa.tile([128, 2, 16, 16], BF16, tag="S")
                            for half in range(2):
                                nc.sync.dma_start(
                                    S[64 * half:64 * half + 64]
                                    .rearrange("c a y x -> c (a y x)"),
                                    Td[:, ts2, :]
                                    .rearrange("c t p -> c (t p)"))
                            for ci, dstd in enumerate((Qd, None, Vd)):
                                ps0 = pap.tile([128, 2, 16, 16], F32,
                                               tag=f"ps{ci}0")
                                ps1 = pap.tile([128, 2, 16, 16], F32,
                                               tag=f"ps{ci}1")
                                for k in range(5):
                                    for ps, offl, rs in (
                                            (ps0, OFFA, slice(0, 64)),
                                            (ps1, OFFB, slice(64, 128))):
                                        dy, dx = offl[k]
                                        liy, loy, cy = _rng(dy)
                                        lix, lox, cx = _rng(dx)
                                        nc.tensor.matmul(
                                            ps[:, :, loy:loy + cy,
                                               lox:lox + cx],
                                            qkvw[rs, ci, k, :],
                                            S[rs, :, liy:liy + cy,
                                              lix:lix + cx],
                                            start=(k == 0), stop=(k == 4),
                                            skip_group_check=True)
                                t0 = pas.tile([128, 512], F32, tag=f"t{ci}")
                                nc.scalar.copy(
                                    t0[:],
                                    ps0.rearrange("c a y x -> c (a y x)"))
                                if ci == 1:
                                    nc.vector.tensor_add(
                                        Ksb[:, ts2, :]
                                        .rearrange("c t p -> c (t p)"),
                                        t0[:],
                                        ps1.rearrange("c a y x -> c (a y x)"))
                                else:
                                    st = pas.tile([128, 512], BF16,
                                                  tag=f"s{ci}")
                                    nc.vector.tensor_add(
                                        st[:], t0[:],
                                        ps1.rearrange("c a y x -> c (a y x)"))
                                    nc.gpsimd.dma_start(
                                        dstd[:, ts2, :]
                                        .rearrange("c t p -> c (t p)"),
                                        st[:])

                    # ---- phase B: scores + softmax (both heads row-tiled) --
                    with tc.tile_pool(name=f"Bs{l}_{rep}", bufs=2) as pbs, \
                         tc.tile_pool(name=f"BQ{l}_{rep}", bufs=1) as pbq, \
                         tc.tile_pool(name=f"Bp{l}_{rep}", bufs=2,
                                      space="PSUM") as pbp, \
                         tc.tile_pool(name=f"Bt{l}_{rep}", bufs=2,
                                      space="PSUM") as pbt:
                        for jb in range(2 if ph_on('B', l) else 0):
                            Qsb = pbq.tile([128, 128, P], BF16, tag="Qsb")
                            nc.sync.dma_start(
                                Qsb[:], Qd[:, 128 * jb:128 * jb + 128, :])
                            sps = [pbp.tile([128, 512], F32, tag=f"sps{h}",
                                             name=f"sps{h}")
                                   for h in range(2)]
                            for p in range(P):
                                for h in range(2):
                                    hs = slice(64 * h, 64 * h + 64)
                                    nc.tensor.matmul(
                                        sps[h][:, 0:T],
                                        Qsb[hs, :, p], Ksb[hs, :, p],
                                        start=(p == 0), stop=(p == P - 1))
                            for h in range(2):
                                nmax = pbs.tile([128, 1], F32, tag=f"nm{h}")
                                nc.vector.reduce_max(
                                    nmax[:], sps[h][:, 0:T],
                                    axis=mybir.AxisListType.X, negate=True)
                                E = pbs.tile([128, T], BF16, tag=f"E{h}")
                                esum = pbs.tile([128, 1], F32, tag=f"es{h}")
                                nc.scalar.activation(E[:], sps[h][:, 0:T],
                                                     AF.Exp,
                                                     bias=nmax[:, 0:1],
                                                     accum_out=esum[:])
                                rinv = pbs.tile([128, 1], F32, tag=f"ri{h}")
                                nc.vector.reciprocal(rinv[:], esum[:])
                                En = pbs.tile([128, T], BF16, tag=f"En{h}")
                                nc.vector.tensor_scalar_mul(
                                    En[:], E[:], rinv[:, 0:1])
                                for qb in range(2):
                                    tps = pbt.tile([128, 1024], BF16,
                                                   tag="tps")
                                    nc.tensor.matmul(
                                        tps[:, 0:128],
                                        En[:, 128 * qb:128 * qb + 128],
                                        ident[:], is_transpose=True)
                                    nc.scalar.copy(
                                        STts[h][:, qb,
                                                128 * jb:128 * jb + 128],
                                        tps[:, 0:128])
                    pbk_cm.__exit__(None, None, None)

                    # ---- phase C: O = softmax(scores) @ V ----
                    with tc.tile_pool(name=f"C{l}_{rep}", bufs=2) as pc, \
                         tc.tile_pool(name=f"Cs{l}_{rep}", bufs=4) as pcs, \
                         tc.tile_pool(name=f"Cp{l}_{rep}", bufs=1,
                                      space="PSUM") as pcp:
                        for h in range(2 if ph_on('C', l) else 0):
                            vts = []
                            for qb in range(2):
                                Vt = pc.tile([128, HID, P], BF16,
                                             tag=f"Vt{qb}")
                                nc.scalar.dma_start(
                                    Vt[:],
                                    Vd[64 * h:64 * h + 64,
                                       128 * qb:128 * qb + 128, :]
                                    .rearrange("c q p -> q c p"))
                                vts.append(Vt)
                            for jb in range(2):
                                for cpg in range(8):
                                    opss = [pcp.tile([128, 512], F32,
                                                     tag=f"ops{i}",
                                                     name=f"ops{i}")
                                            for i in range(4)]
                                    for qb in range(2):
                                        for i in range(4):
                                            cp = 4 * cpg + i
                                            nc.tensor.matmul(
                                                opss[i][:],
                                                STts[h][:, qb,
                                                        128 * jb:128 * jb + 128],
                                                vts[qb][:, 2 * cp:2 * cp + 2, :]
                                                .rearrange("q c p -> q (c p)"),
                                                start=(qb == 0),
                                                stop=(qb == 1))
                                    for i in range(4):
                                        cp = 4 * cpg + i
                                        ost = pcs.tile([128, 512], BF16,
                                                       tag="ost")
                                        nc.scalar.copy(ost[:], opss[i][:])
                                        nc.gpsimd.dma_start(
                                            Od[64 * h + 2 * cp:
                                               64 * h + 2 * cp + 2,
                                               128 * jb:128 * jb + 128, :]
                                            .rearrange("c j p -> j c p"),
                                            ost.rearrange("j (c p) -> j c p",
                                                          c=2))
                    pbst_cm.__exit__(None, None, None)

                    # ---- phase D: unify conv (col-tiled pairs) + RS ----
                    Ud = [dram.tile([4, HID, 16, P], BF16,
                                    tag=f"Ud{q}_{rep}", name=f"Ud{q}")
                          for q in range(4)]
                    RSo = [dram.tile([HID, 16, P], BF16,
                                     tag=f"RSo{q}_{rep}", name=f"RSo{q}")
                           for q in range(4)]
                    with tc.tile_pool(name=f"D{l}_{rep}", bufs=3) as pd, \
                         tc.tile_pool(name=f"Dw{l}_{rep}", bufs=1) as pdw, \
                         tc.tile_pool(name=f"Dp{l}_{rep}", bufs=3,
                                      space="PSUM") as pdp:
                        uw = pdw.tile([2 * HID, 9, HID], BF16, tag="uw")
                        if ph_on('D', l):
                            nc.sync.dma_start(uw[:], uwT_in.ap()[l])
                        for i in range(T // 4 if ph_on('D', l) else 0):
                            g0 = 4 * i
                            q, r = g0 // 64, (g0 % 64) // 16
                            s = g0 % 16
                            och = pd.tile([128, 4, 16, 16], BF16, tag="och")
                            nc.sync.dma_start(
                                och.rearrange("c a y x -> c (a y x)"),
                                Od[:, g0:g0 + 4, :]
                                .rearrange("c t p -> c (t p)"))
                            tsl = pd.tile([128, 2, P], BF16, tag="tsl")
                            for half in range(2):
                                nc.sync.dma_start(
                                    tsl[64 * half:64 * half + 64]
                                    .rearrange("c t p -> c (t p)"),
                                    Td[:, g0 + 2 * half:g0 + 2 * half + 2, :]
                                    .rearrange("c t p -> c (t p)"))
                            ups = pdp.tile([128, 2, 16, 16], F32, tag="ups")
                            for d, (dy, dx) in enumerate(OFFS):
                                liy, loy, cy = _rng(dy)
                                lix, lox, cx = _rng(dx)
                                for cg in range(2):
                                    nc.tensor.matmul(
                                        ups[64 * cg:64 * cg + 64, :,
                                            loy:loy + cy, lox:lox + cx],
                                        uw[:, d, :],
                                        och[:, 2 * cg:2 * cg + 2,
                                            liy:liy + cy, lix:lix + cx],
                                        start=(d == 0), stop=(d == 8),
                                        skip_group_check=True)
                            ust = pd.tile([128, 2 * P], BF16, tag="ust")
                            nc.vector.scalar_tensor_tensor(
                                ust[:], tsl.rearrange("c t p -> c (t p)"),
                                sel_sb[:, r:r + 1],
                                ups.rearrange("c a y x -> c (a y x)"),
                                op0=ALU.mult, op1=ALU.add)
                            for half in range(2):
                                nc.gpsimd.dma_start(
                                    Ud[q][r, :, s + 2 * half:s + 2 * half + 2,
                                          :].rearrange("c t p -> c (t p)"),
                                    ust[64 * half:64 * half + 64, :])
                            if g0 % 64 == 60 and ph_on('R', l):
                                nc.gpsimd.collective_compute(
                                    "ReduceScatter", ALU.add,
                                    replica_groups=GROUPS,
                                    ins=[Ud[q].opt()], outs=[RSo[q].opt()])

                    # ---- phase E: LN1 + MLP + LN2, two quarters/round ----
                    AGi = [dram.tile([HID, 16, P], BF16,
                                     tag=f"AGi{q}_{rep}", name=f"AGi{q}")
                           for q in range(4)]
                    AGo = [dram.tile([4, HID, 16, P], BF16,
                                     tag=f"AGo{q}_{rep}", name=f"AGo{q}")
                           for q in range(4)]
                    with tc.tile_pool(name=f"E{l}_{rep}", bufs=1) as pe, \
                         tc.tile_pool(name=f"Es{l}_{rep}", bufs=2) as pes, \
                         tc.tile_pool(name=f"Ew{l}_{rep}", bufs=1) as pew, \
                         tc.tile_pool(name=f"Ep{l}_{rep}", bufs=1,
                                      space="PSUM") as pep, \
                         tc.tile_pool(name=f"Ep2{l}_{rep}", bufs=2,
                                      space="PSUM") as pep2:
                        m1w = pew.tile([128, 2, 128], BF16, tag="m1w")
                        m1bs = pew.tile([2 * HID, 2], F32, tag="m1bs")
                        m2w = pew.tile([2 * HID, 2, HID], BF16, tag="m2w")
                        m2bs = pew.tile([128, 1], F32, tag="m2bs")
                        ln_w = pew.tile([128, 2, P], F32, tag="ln_w")
                        ln_b = pew.tile([128, 2, P], F32, tag="ln_b")
                        if ph_on('E', l):
                            nc.sync.dma_start(m1w[:], m1w2_in.ap()[l])
                            nc.sync.dma_start(m1bs[:], m1b_in.ap()[l])
                            nc.sync.dma_start(m2w[:], m2T_in.ap()[l])
                            nc.sync.dma_start(m2bs[:], m2b2_in.ap()[l])
                            for half in range(2):
                                hs = slice(64 * half, 64 * half + 64)
                                nc.sync.dma_start(
                                    ln_w[hs], lnw_in.ap()[l]
                                    .rearrange("w c p -> c w p"))
                                nc.sync.dma_start(
                                    ln_b[hs], lnb_in.ap()[l]
                                    .rearrange("w c p -> c w p"))

                        def layernorm(xin, wi, out_b, out_f=None):
                            xsq = pe.tile([128, 16, P], F32, tag="xsq")
                            nc.gpsimd.tensor_mul(xsq[:], xin[:], xin[:])
                            stats = pes.tile([128, 32], F32, tag="stats")
                            nc.vector.reduce_sum(stats[:, 0:16], xin[:],
                                                 axis=mybir.AxisListType.X)
                            nc.vector.reduce_sum(stats[:, 16:32], xsq[:],
                                                 axis=mybir.AxisListType.X)
                            sp = pep2.tile([128, 512], F32, tag="sp")
                            nc.tensor.matmul(sp[:, 0:32], onesblk[:],
                                             stats[:], start=True, stop=True)
                            mu = pes.tile([128, 48], F32, tag="mu")
                            nc.scalar.mul(mu[:, 0:16], sp[:, 0:16],
                                          1.0 / 16384)
                            nc.scalar.mul(mu[:, 16:32], sp[:, 16:32],
                                          1.0 / 16384)
                            nc.scalar.square(mu[:, 32:48], mu[:, 0:16])
                            var = pes.tile([128, 32], F32, tag="var")
                            nc.vector.tensor_sub(var[:, 0:16], mu[:, 16:32],
                                                 mu[:, 32:48])
                            nc.scalar.activation(var[:, 16:32], var[:, 0:16],
                                                 AF.Sqrt, bias=eps_sb[:, 0:1])
                            rstd = pes.tile([128, 16], F32, tag="rstd")
                            nc.vector.reciprocal(rstd[:], var[:, 16:32])
                            mu_bc = mu[:, 0:16].unsqueeze(-1) \
                                .broadcast_to((128, 16, P))
                            rs_bc = rstd[:, 0:16].unsqueeze(-1) \
                                .broadcast_to((128, 16, P))
                            w_bc = ln_w[:, wi, :].unsqueeze(1).broadcast_to(
                                (128, 16, P))
                            b_bc = ln_b[:, wi, :].unsqueeze(1).broadcast_to(
                                (128, 16, P))
                            xn = pe.tile([128, 16, P], F32, tag="xn")
                            nc.gpsimd.tensor_sub(xn[:], xin[:], mu_bc)
                            nc.gpsimd.tensor_mul(xn[:], xn[:], rs_bc)
                            nc.gpsimd.tensor_mul(xn[:], xn[:], w_bc)
                            if out_f is not None:
                                nc.gpsimd.tensor_add(out_f[:], xn[:], b_bc)
                                nc.vector.tensor_copy(out_b[:], out_f[:])
                            else:
                                nc.gpsimd.tensor_add(out_b[:], xn[:], b_bc)

                        for rr in range(2 if ph_on('E', l) else 0):
                            xi = pe.tile([128, 16, P], BF16, tag="xi")
                            for half in range(2):
                                hs = slice(64 * half, 64 * half + 64)
                                nc.sync.dma_start(xi[hs], RSo[2 * rr + half])
                            idn_f = pe.tile([128, 16, P], F32, tag="idn_f")
                            idn_b = pe.tile([128, 16, P], BF16, tag="idn_b")
                            layernorm(xi, 0, idn_b, idn_f)
                            m1sb = [pe.tile([128, 2, 16, P], BF16,
                                            tag=f"m1sb{qq}",
                                            name=f"m1sb{qq}")
                                    for qq in range(2)]
                            for c2 in range(8):
                                cs = slice(2 * c2, 2 * c2 + 2)
                                mps = {}
                                for mb in range(2):
                                    for qq in range(2):
                                        qs = slice(64 * qq, 64 * qq + 64)
                                        mp = pep.tile([128, 512], F32,
                                                      tag=f"mp{qq}{mb}")
                                        nc.tensor.matmul(
                                            mp[:], m1w[qs, mb, :],
                                            idn_b[qs, cs, :]
                                            .rearrange("c t p -> c (t p)"),
                                            start=True, stop=True)
                                        mps[(qq, mb)] = mp
                                for mb in range(2):
                                    for qq in range(2):
                                        nc.scalar.activation(
                                            m1sb[qq][:, mb, cs, :]
                                            .rearrange("c t p -> c (t p)"),
                                            mps[(qq, mb)][:], AF.Relu,
                                            bias=m1bs[:, mb:mb + 1])
                            x2f = pe.tile([128, 16, P], F32, tag="x2f")
                            for c2 in range(8):
                                cs = slice(2 * c2, 2 * c2 + 2)
                                mp2 = pep2.tile([128, 512], F32, tag="mp2")
                                for mb in range(2):
                                    for qq in range(2):
                                        nc.tensor.matmul(
                                            mp2[64 * qq:64 * qq + 64, :],
                                            m2w[:, mb, :],
                                            m1sb[qq][:, mb, cs, :]
                                            .rearrange("c t p -> c (t p)"),
                                            start=(mb == 0), stop=(mb == 1))
                                nc.vector.scalar_tensor_tensor(
                                    x2f[:, cs, :]
                                    .rearrange("c t p -> c (t p)"),
                                    mp2[:], m2bs[:, 0:1],
                                    idn_f[:, cs, :]
                                    .rearrange("c t p -> c (t p)"),
                                    op0=ALU.add, op1=ALU.add)
                            yb = pe.tile([128, 16, P], BF16, tag="yb")
                            layernorm(x2f, 1, yb)
                            for half in range(2):
                                q = 2 * rr + half
                                hs = slice(64 * half, 64 * half + 64)
                                nc.sync.dma_start(AGi[q][:], yb[hs])
                                if ph_on('R', l):
                                    nc.gpsimd.collective_compute(
                                        "AllGather", ALU.bypass,
                                        replica_groups=GROUPS,
                                        ins=[AGi[q].opt()],
                                        outs=[AGo[q].opt()])
                    # write gathered T back to Td (DRAM->DRAM)
                    for q in range(4 if ph_on('R', l) else 0):
                        nc.sync.dma_start(
                            Td[:, 64 * q:64 * q + 64, :]
                            .rearrange("c (r s) p -> c r s p", r=4),
                            AGo[q].rearrange("r c s p -> c r s p"))

                # ---------- output head (rank-symmetric tile-row bands) ----
                with tc.tile_pool(name=f"H{rep}", bufs=2) as ph, \
                     tc.tile_pool(name=f"Hw{rep}", bufs=1) as phw, \
                     tc.tile_pool(name=f"Hp{rep}", bufs=2, space="PSUM") as php:
                    ow1 = phw.tile([128, 5, HID], BF16, tag="ow1")
                    ob1 = phw.tile([HID, 1], F32, tag="ob1")
                    ow2 = phw.tile([HID, 4], BF16, tag="ow2")
                    ob2 = phw.tile([4, 1], F32, tag="ob2")
                    if ph_on('H'):
                        nc.sync.dma_start(ow1[:], outw1b2_in.ap())
                        nc.sync.dma_start(ob1[:], outb1_in.ap())
                        nc.sync.dma_start(ow2[:], outw2T_in.ap())
                        nc.sync.dma_start(ob2[:], outb2_in.ap())
                    for q in range(4 if ph_on('H') else 0):
                        rows = ph.tile([128, 6, 16, P], BF16, tag="rows",
                                       bufs=1)
                        for j in range(6):
                            m = 4 * q - 1 + j
                            if 0 <= m <= 15:
                                for half in range(2):
                                    hs = slice(64 * half, 64 * half + 64)
                                    nc.sync.dma_start(
                                        rows[hs, j]
                                        .rearrange("c t p -> c (t p)"),
                                        Td[:, 16 * m:16 * m + 16, :]
                                        .rearrange("c t p -> c (t p)"))
                            else:
                                nc.gpsimd.memset(
                                    rows[:, j].rearrange("c t p -> c (t p)"),
                                    0.0)
                        img = ph.tile([128, 18, 16, 16], BF16, tag="img",
                                      bufs=1)
                        nc.gpsimd.memset(
                            img.rearrange("c a t x -> c (a t x)"), 0.0)
                        rows4 = rows.rearrange("c j t (y x) -> c j t y x",
                                               y=16)
                        for r in range(4):
                            sel = sel_sb[:, r:r + 1]
                            segs = [(img[:, 0, :, :], rows4[:, r, :, 15, :])]
                            for y in range(16):
                                segs.append((img[:, 1 + y, :, :],
                                             rows4[:, r + 1, :, y, :]))
                            segs.append((img[:, 17, :, :],
                                         rows4[:, r + 2, :, 0, :]))
                            for dst, srcap in segs:
                                nc.vector.scalar_tensor_tensor(
                                    dst, srcap, sel, dst,
                                    op0=ALU.mult, op1=ALU.add)
                        imgf = img.rearrange("c a t x -> c a (t x)")
                        for ch in range(8):
                            oc0 = php.tile([HID, 2, 256], F32, tag="oc0")
                            oc1 = php.tile([HID, 2, 256], F32, tag="oc1")
                            for k in range(5):
                                for ps, offl, rs in (
                                        (oc0, OFFA, slice(0, 64)),
                                        (oc1, OFFB, slice(64, 128))):
                                    dy, dx = offl[k]
                                    lix, lox, cx = _rng(dx, 256)
                                    nc.tensor.matmul(
                                        ps[:, 0:2, lox:lox + cx],
                                        ow1[rs, k, :],
                                        imgf[rs, 2 * ch + dy:2 * ch + dy + 2,
                                             lix:lix + cx],
                                        start=(k == 0), stop=(k == 4),
                                        skip_group_check=True)
                            tt = ph.tile([HID, 2 * 256], F32, tag="tt")
                            nc.scalar.copy(
                                tt[:], oc0.rearrange("c a x -> c (a x)"))
                            o1 = ph.tile([HID, 2 * 256], BF16, tag="o1")
                            nc.vector.tensor_add(
                                o1[:], tt[:],
                                oc1.rearrange("c a x -> c (a x)"))
                            o1r = ph.tile([HID, 2 * 256], BF16, tag="o1r")
                            nc.scalar.activation(o1r[:], o1[:], AF.Relu,
                                                 bias=ob1[:, 0:1])
                            p2 = php.tile([4, 2 * 256], F32, tag="p2")
                            nc.tensor.matmul(p2[:], ow2[:], o1r[:],
                                             start=True, stop=True)
                            ysb = ph.tile([4, 2 * 256], F32, tag="ysb")
                            nc.vector.tensor_scalar_add(ysb[:], p2[:],
                                                        ob2[:, 0:1])
                            nc.sync.dma_start(
                                y_out.ap()[:, q, 2 * ch:2 * ch + 2, :]
                                .rearrange("c t x -> c (t x)"), ysb[:])
    nc.finalize()
    return nc, taps


# ======================= host side =======================

def _prep_core_inputs(core, inputs):
    r = core % 4
    b = core // 4
    f32 = np.float32
    bf = lambda a: np.ascontiguousarray(np.asarray(a, f32)).astype(NBF)
    x = np.asarray(inputs["x"], f32)
    xtc = x[b].reshape(3, 16, 16, 16, 16).transpose(0, 1, 3, 2, 4) \
        .reshape(3, T, P)
    pos = np.asarray(inputs["pos"], f32).reshape(T, HID).T.copy()
    hsl = slice(2 * r * HID, 2 * r * HID + 2 * HID)

    offa_i = [dy * 3 + dx for dy, dx in OFFA]
    offb_i = [dy * 3 + dx for dy, dx in OFFB]
    qkv2 = np.empty((L, 128, 3, 5, 128), f32)
    for l in range(L):
        for i, (nm, sc) in enumerate((("qw", 0.25), ("kw", 0.25),
                                      ("vw", 1.0))):
            w = np.asarray(inputs[nm], f32)[l, hsl] * sc  # [128,64,3,3]
            wt = w.transpose(1, 2, 3, 0).reshape(HID, 9, 2 * HID)
            for k in range(5):
                sca = 0.5 if k == 0 else 1.0
                qkv2[l, 0:64, i, k, :] = wt[:, offa_i[k], :] * sca
                qkv2[l, 64:128, i, k, :] = wt[:, offb_i[k], :] * sca
    offs_i = [dy * 3 + dx for dy, dx in OFFS]
    uwp = np.empty((L, 2 * HID, 9, HID), f32)
    for l in range(L):
        w = np.asarray(inputs["uw"], f32)[l][:, hsl]      # [64,128,3,3]
        wt = w.transpose(1, 2, 3, 0).reshape(2 * HID, 9, HID)
        uwp[l] = wt[:, offs_i, :]
    m1 = np.asarray(inputs["mlp_w1"], f32)[:, :, :, 0, 0]  # [L,256,64]
    m1T = m1.transpose(0, 2, 1).reshape(L, HID, 2, 128)    # [L,64,2,128]
    m1w2 = np.concatenate([m1T, m1T], axis=1)              # [L,128,2,128]
    m1bp = np.asarray(inputs["mlp_b1"], f32).reshape(L, 2, 2 * HID) \
        .transpose(0, 2, 1).copy()                         # [L,128,2]
    m2 = np.asarray(inputs["mlp_w2"], f32)[:, :, :, 0, 0]  # [L,64,256]
    m2Tp = m2.transpose(0, 2, 1).reshape(L, 2, 2 * HID, HID) \
        .transpose(0, 2, 1, 3).copy()                      # [L,128,2,64]
    m2b = np.asarray(inputs["mlp_b2"], f32)                # [L,64]
    m2b2 = np.concatenate([m2b, m2b], axis=1).reshape(L, 128, 1)
    lnwp = np.stack([np.asarray(inputs["ln1_w"], f32).reshape(L, HID, P),
                     np.asarray(inputs["ln2_w"], f32).reshape(L, HID, P)], 1)
    lnbp = np.stack([np.asarray(inputs["ln1_b"], f32).reshape(L, HID, P),
                     np.asarray(inputs["ln2_b"], f32).reshape(L, HID, P)], 1)
    ow1 = np.asarray(inputs["out_w1"], f32)                # [64,64,3,3]
    ow1T = ow1.transpose(1, 2, 3, 0).reshape(HID, 9, HID)
    ow1b2 = np.empty((128, 5, HID), f32)
    for k in range(5):
        sca = 0.5 if k == 0 else 1.0
        ow1b2[0:64, k, :] = ow1T[:, offa_i[k], :] * sca
        ow1b2[64:128, k, :] = ow1T[:, offb_i[k], :] * sca
    ow2 = np.asarray(inputs["out_w2"], f32)[:, :, 0, 0]    # [3,64]
    ow2T = np.zeros((HID, 4), f32)
    ow2T[:, :3] = ow2.T
    ob2 = np.zeros((4, 1), f32)
    ob2[:3, 0] = np.asarray(inputs["out_b2"], f32)
    sel = np.zeros((128, 4), f32)
    sel[0:64, r] = 1.0
    sel[64:128, r] = 1.0
    onesblk = np.zeros((128, 128), f32)
    onesblk[0:64, 0:64] = 1.0
    onesblk[64:128, 64:128] = 1.0

    return {
        "xt": bf(xtc), "pos_in": pos.astype(np.float32),
        "semwT": bf(np.asarray(inputs["sem_w"], f32)[:, :, 0, 0].T),
        "semb": np.asarray(inputs["sem_b"], f32).reshape(HID, 1).copy(),
        "qkvw2": bf(qkv2), "uwT": bf(uwp), "m1w2": bf(m1w2), "m1b": m1bp,
        "m2T": bf(m2Tp), "m2b2": m2b2, "lnw": lnwp, "lnb": lnbp,
        "outw1b2": bf(ow1b2),
        "outb1": np.asarray(inputs["out_b1"], f32).reshape(HID, 1).copy(),
        "outw2T": bf(ow2T), "outb2": ob2,
        "ident_in": np.eye(128, dtype=NBF), "sel_in": sel,
        "onesblk_in": onesblk,
    }


def assemble_output(results):
    img = np.zeros((B, 3, 256, 256), np.float32)
    for c in range(N_CORES):
        b, r = c // 4, c % 4
        y = np.asarray(results[c]["y_out"], np.float32)  # [4,4,16,256]
        for q in range(4):
            rb = 16 * (4 * q + r)
            img[b, :, rb:rb + 16, :] = y[:3, q]
    return img


_CACHE = {}


def get_built(debug_taps=()):
    key = tuple(sorted(debug_taps))
    if key not in _CACHE:
        t0 = time.time()
        nc, taps = build(debug_taps)
        _CACHE[key] = (nc, taps)
        print(f"[kernel] build {time.time() - t0:.1f}s", file=sys.stderr)
    return _CACHE[key]


def run_cores(inputs, debug_taps=()):
    from concourse import bass2jax
    nc, taps = get_built(debug_taps)
    in_maps = [_prep_core_inputs(c, inputs) for c in range(N_CORES)]
    t0 = time.time()
    results = bass2jax.run_bass_via_pjrt(nc, in_maps, n_cores=N_CORES)
    print(f"[kernel] run {time.time() - t0:.1f}s", file=sys.stderr)
    return results


def kernel(**inputs):
    results = run_cores(inputs)
    return assemble_output(results)


# revision 25
# speedup vs baseline: 1.8791x; 1.3642x over previous
"""nn_CNNTransformer Trainium2 kernel — full-input/full-output contract.

Sharding (8 NeuronCores): 2 batch groups x 4 cores.  Within a group each
core computes the QKV convs + attention + partial unify conv for its 2 of
the 8 heads; unify partials (with the residual folded in on exactly one
rank via a one-hot multiplier) are ReduceScattered (bf16) over the group
in 4 tile-quarter chunks, LayerNorm+MLP+LayerNorm run on the core's tile
shard (two quarters at a time across the 128 partitions), and chunked
AllGathers rebuild the full activation for the next layer.

v2: all K=64 contractions are row-tiled across the two 64-partition
halves of the PE array (two concurrent matmuls in different row groups,
two PSUM banks, DVE add at eviction) and 64-col outputs are col-tiled —
this keeps the PE at the warm 2.4 GHz clock and roughly doubles matmul
throughput.  Activations live in DRAM as bf16.

Self-contained: only needs the concourse tree at /opt/trn_rl_repo.
"""
import os
import sys
import time

for _p in ("/opt/trn_rl_repo", "/root/.axon_site/_ro/trn_rl_repo"):
    if os.path.isdir(_p) and _p not in sys.path:
        sys.path.insert(0, _p)
        break

import numpy as np
import ml_dtypes

import concourse.bacc as bacc
import concourse.mybir as mybir
import concourse.tile as tile
B, NT, HID, HEADS, L = 2, 16, 64, 8, 2
T = NT * NT            # 256 tiles
P = 256                # pixels per 16x16 tile
N_CORES = 8
GROUPS = [[0, 1, 2, 3], [4, 5, 6, 7]]
BF16 = mybir.dt.bfloat16
F32 = mybir.dt.float32
NBF = ml_dtypes.bfloat16
AF = mybir.ActivationFunctionType
ALU = mybir.AluOpType

# 3x3 offsets, center first (full-region matmul opens each PSUM bank).
OFFS = [(1, 1)] + [(dy, dx) for dy in range(3) for dx in range(3)
                   if (dy, dx) != (1, 1)]
# bank0 gets half-center + OFFS[1,3,5,7]; bank1 half-center + OFFS[2,4,6,8]
OFFA = [OFFS[i] for i in (0, 1, 3, 5, 7)]
OFFB = [OFFS[i] for i in (0, 2, 4, 6, 8)]


def _rng(o, n=16):
    return max(o - 1, 0), max(1 - o, 0), n - abs(o - 1)


def build(debug_taps=(), only=None, reps=1):
    def ph_on(p, l=None):
        if only is None:
            return True
        return p in only or (l is not None and f"{p}{l}" in only)
    nc = bacc.Bacc(None, target_bir_lowering=False, debug=False)

    xt = nc.dram_tensor("xt", [3, T, P], BF16, kind="ExternalInput")
    pos_in = nc.dram_tensor("pos_in", [HID, T], F32, kind="ExternalInput")
    semwT = nc.dram_tensor("semwT", [3, HID], BF16, kind="ExternalInput")
    semb = nc.dram_tensor("semb", [HID, 1], F32, kind="ExternalInput")
    qkvw2_in = nc.dram_tensor("qkvw2", [L, 128, 3, 5, 128], BF16,
                              kind="ExternalInput")
    uwT_in = nc.dram_tensor("uwT", [L, 2 * HID, 9, HID], BF16,
                            kind="ExternalInput")
    m1w2_in = nc.dram_tensor("m1w2", [L, 128, 2, 128], BF16,
                             kind="ExternalInput")
    m1b_in = nc.dram_tensor("m1b", [L, 2 * HID, 2], F32,
                            kind="ExternalInput")
    m2T_in = nc.dram_tensor("m2T", [L, 2 * HID, 2, HID], BF16,
                            kind="ExternalInput")
    m2b2_in = nc.dram_tensor("m2b2", [L, 128, 1], F32, kind="ExternalInput")
    lnw_in = nc.dram_tensor("lnw", [L, 2, HID, P], F32, kind="ExternalInput")
    lnb_in = nc.dram_tensor("lnb", [L, 2, HID, P], F32, kind="ExternalInput")
    outw1b2_in = nc.dram_tensor("outw1b2", [128, 5, HID], BF16,
                                kind="ExternalInput")
    outb1_in = nc.dram_tensor("outb1", [HID, 1], F32, kind="ExternalInput")
    outw2T_in = nc.dram_tensor("outw2T", [HID, 4], BF16,
                               kind="ExternalInput")
    outb2_in = nc.dram_tensor("outb2", [4, 1], F32, kind="ExternalInput")
    ident_in = nc.dram_tensor("ident_in", [128, 128], BF16,
                              kind="ExternalInput")
    sel_in = nc.dram_tensor("sel_in", [128, 14], F32, kind="ExternalInput")
    onesblk_in = nc.dram_tensor("onesblk_in", [128, 128], F32,
                                kind="ExternalInput")
    y_out = nc.dram_tensor("y_out", [4, 4, 16, NT * 16], F32,
                           kind="ExternalOutput")

    taps = {}

    def tap(name, shape, dtype=BF16):
        if name in debug_taps and name not in taps:
            taps[name] = nc.dram_tensor("tap_" + name, shape, dtype,
                                        kind="ExternalOutput")
        return taps.get(name)

    with tile.TileContext(nc) as tc:
        with tc.tile_pool(name="dram", bufs=1, space="DRAM") as dram, \
             tc.tile_pool(name="persist", bufs=1) as persist:

            ident = persist.tile([128, 128], BF16, tag="ident")
            onesblk = persist.tile([128, 128], F32, tag="onesblk")
            sel_sb = persist.tile([128, 14], F32, tag="sel_sb")
            eps_sb = persist.tile([128, 1], F32, tag="eps_sb")
            nc.sync.dma_start(ident[:], ident_in.ap())
            nc.sync.dma_start(onesblk[:], onesblk_in.ap())
            nc.sync.dma_start(sel_sb[:], sel_in.ap())
            nc.gpsimd.memset(eps_sb[:], 1e-5)

            for rep in range(reps):
                Qd = dram.tile([128, T, P], BF16, tag=f"Qd{rep}", name="Qd")
                Vd = dram.tile([128, T, P], BF16, tag=f"Vd{rep}", name="Vd")
                Od = dram.tile([128, T, P], BF16, tag=f"Od{rep}", name="Od")
                Td = dram.tile([HID, T, P], BF16, tag=f"Td{rep}", name="Td")
                StripI = dram.tile([HID, 4, 2, 16, 16], BF16,
                                   tag=f"StripI{rep}", name="StripI")
                StripO = dram.tile([4, HID, 4, 2, 16, 16], BF16,
                                   tag=f"StripO{rep}", name="StripO")
                # ---------- stage 0: sem 1x1 conv + ReLU + pos -> Td -------
                with tc.tile_pool(name=f"s0{rep}", bufs=3) as s0, \
                     tc.tile_pool(name=f"s0w{rep}", bufs=1) as s0w, \
                     tc.tile_pool(name=f"s0p{rep}", bufs=2, space="PSUM") as s0p:
                    swt = s0w.tile([128, HID], BF16, tag="swt")
                    sbt = s0w.tile([HID, 1], F32, tag="sbt")
                    posf_sb = s0w.tile([HID, T], F32, tag="posf_sb")
                    nc.sync.dma_start(posf_sb[:], pos_in.ap())
                    if ph_on('S'):
                        nc.sync.dma_start(swt[0:3, :], semwT.ap())
                        nc.sync.dma_start(swt[64:67, :], semwT.ap())
                        nc.sync.dma_start(sbt[:], semb.ap())
                    for i in range(T // 8 if ph_on('S') else 0):
                        g0 = 8 * i
                        xs = s0.tile([128, 4, P], BF16, tag="xs")
                        nc.sync.dma_start(
                            xs[0:3, :, :].rearrange("c t p -> c (t p)"),
                            xt.ap()[:, g0:g0 + 4, :]
                            .rearrange("c t p -> c (t p)"))
                        nc.sync.dma_start(
                            xs[64:67, :, :].rearrange("c t p -> c (t p)"),
                            xt.ap()[:, g0 + 4:g0 + 8, :]
                            .rearrange("c t p -> c (t p)"))
                        pss = []
                        for u in range(4):
                            half = u // 2
                            hs = slice(64 * half, 64 * half + 3)
                            ts = slice(2 * (u % 2), 2 * (u % 2) + 2)
                            ps = s0p.tile([HID, 2 * P], F32, tag=f"ps{u}",
                                          name=f"ps{u}")
                            nc.tensor.matmul(
                                ps[:], swt[hs, :],
                                xs[hs, ts, :].rearrange("c t p -> c (t p)"),
                                start=True, stop=True)
                            pss.append(ps)
                        for u, ps in enumerate(pss):
                            g = g0 + 2 * u
                            tch = s0.tile([HID, 2, P], F32, tag=f"tch{u}",
                                          name="tch")
                            nc.scalar.activation(
                                tch.rearrange("c t p -> c (t p)"), ps[:],
                                AF.Relu, bias=sbt[:, 0:1])
                            ob = s0.tile([HID, 2, P], BF16, tag=f"ob{u}",
                                         name="ob")
                            eng = nc.vector if u % 2 == 0 else nc.gpsimd
                            eng.tensor_add(
                                ob[:], tch[:],
                                posf_sb[:, g:g + 2].unsqueeze(-1)
                                .broadcast_to((HID, 2, P)))
                            nc.sync.dma_start(
                                Td[:, g:g + 2, :]
                                .rearrange("c t p -> c (t p)"),
                                ob.rearrange("c t p -> c (t p)"))

                # ---------- transformer layers ----------
                for l in range(L):
                    # ---- phase A: QKV convs, row-tiled halves ----
                    pbst_cm = tc.tile_pool(name=f"Bst{l}_{rep}", bufs=1)
                    pbst = pbst_cm.__enter__()
                    STts = [pbst.tile([128, 2, T], BF16, tag=f"STt{h}",
                                      name=f"STt{h}") for h in range(2)]
                    paw_cm = tc.tile_pool(name=f"Aw{l}_{rep}", bufs=1)
                    paw = paw_cm.__enter__()
                    qkvw = paw.tile([128, 3, 5, 128], BF16, tag="qkvw")
                    if ph_on('A', l) or ph_on('B', l):
                        nc.sync.dma_start(qkvw[:], qkvw2_in.ap()[l])
                    pbk_cm = tc.tile_pool(name=f"BK{l}_{rep}", bufs=1)
                    pbk = pbk_cm.__enter__()
                    Ksb = pbk.tile([128, T, P], BF16, tag="Ksb")

                    stg = {}

                    def conv_pair(tp, ci, pool, pspool, dstd, S=None):
                        """3x3 conv for tile-pair tp of conv ci, row-tiled."""
                        ts2 = slice(2 * tp, 2 * tp + 2)
                        if S is None:
                            S = pool.tile([128, 2, 16, 16], BF16,
                                          tag=f"S{ci}", name="S")
                            for half in range(2):
                                nc.sync.dma_start(
                                    S[64 * half:64 * half + 64]
                                    .rearrange("c a y x -> c (a y x)"),
                                    Td[:, ts2, :]
                                    .rearrange("c t p -> c (t p)"))
                        ps0 = pspool.tile([128, 2, 16, 16], F32,
                                          tag=f"cps{ci}0", name="cps0")
                        ps1 = pspool.tile([128, 2, 16, 16], F32,
                                          tag=f"cps{ci}1", name="cps1")
                        for k in range(5):
                            for ps, offl, rs in (
                                    (ps0, OFFA, slice(0, 64)),
                                    (ps1, OFFB, slice(64, 128))):
                                dy, dx = offl[k]
                                liy, loy, cy = _rng(dy)
                                lix, lox, cx = _rng(dx)
                                nc.tensor.matmul(
                                    ps[:, :, loy:loy + cy, lox:lox + cx],
                                    qkvw[rs, ci, k, :],
                                    S[rs, :, liy:liy + cy, lix:lix + cx],
                                    start=(k == 0), stop=(k == 4),
                                    skip_group_check=True)
                        if ci == 2:
                            st = stg["p"].tile([128, 512], BF16,
                                               tag="s2", name="st")
                            nc.scalar.copy(
                                st[:], ps0.rearrange("c a y x -> c (a y x)"))
                            nc.vector.tensor_add(
                                st[:], st[:],
                                ps1.rearrange("c a y x -> c (a y x)"))
                            nc.gpsimd.dma_start(
                                dstd[:, ts2, :].rearrange("c t p -> c (t p)"),
                                st[:])
                            return
                        t0 = stg["p"].tile([128, 512], F32, tag=f"t{ci}",
                                           name="t0")
                        nc.scalar.copy(
                            t0[:], ps0.rearrange("c a y x -> c (a y x)"))
                        if dstd is None:
                            nc.vector.tensor_add(
                                Ksb[:, ts2, :].rearrange("c t p -> c (t p)"),
                                t0[:], ps1.rearrange("c a y x -> c (a y x)"))
                        else:
                            st = stg["p"].tile([128, 512], BF16,
                                               tag="s0", name="st")
                            nc.vector.tensor_add(
                                st[:], t0[:],
                                ps1.rearrange("c a y x -> c (a y x)"))
                            nc.gpsimd.dma_start(
                                dstd[:, ts2, :].rearrange("c t p -> c (t p)"),
                                st[:])

                    pbq_cm = tc.tile_pool(name=f"BQ{l}_{rep}", bufs=1)
                    pbq = pbq_cm.__enter__()
                    Qsb = pbq.tile([128, 128, P], BF16, tag="Qsb")
                    with tc.tile_pool(name=f"A{l}_{rep}", bufs=3) as pa, \
                         tc.tile_pool(name=f"As{l}_{rep}", bufs=1) as pas, \
                         tc.tile_pool(name=f"Ap{l}_{rep}", bufs=1,
                                      space="PSUM") as pap:
                        stg["p"] = pas
                        for tp in range(T // 2 if ph_on('A', l) else 0):
                            ts2 = slice(2 * tp, 2 * tp + 2)
                            S = pa.tile([128, 2, 16, 16], BF16, tag="S",
                                        name="S")
                            for half in range(2):
                                nc.sync.dma_start(
                                    S[64 * half:64 * half + 64]
                                    .rearrange("c a y x -> c (a y x)"),
                                    Td[:, ts2, :]
                                    .rearrange("c t p -> c (t p)"))
                            conv_pair(tp, 0, pa, pap, Qd, S=S)
                            conv_pair(tp, 1, pa, pap, None, S=S)
                            conv_pair(tp, 2, pa, pap, Vd, S=S)
                            if tp == 67 and ph_on('B', l):
                                nc.sync.dma_start(
                                    Qsb[:], Qd[:, 0:128, :])

                    # ---- phase B: scores + softmax (both heads row-tiled) --
                    with tc.tile_pool(name=f"Bs{l}_{rep}", bufs=2) as pbs, \
                         tc.tile_pool(name=f"Bp{l}_{rep}", bufs=2,
                                      space="PSUM") as pbp, \
                         tc.tile_pool(name=f"Bt{l}_{rep}", bufs=2,
                                      space="PSUM") as pbt:
                        for jb in range(2 if ph_on('B', l) else 0):
                            if jb == 1:
                                nc.sync.dma_start(
                                    Qsb[:],
                                    Qd[:, 128 * jb:128 * jb + 128, :])
                            sps = [pbp.tile([128, 512], F32, tag=f"sps{h}",
                                             name=f"sps{h}")
                                   for h in range(2)]
                            for p in range(P):
                                for h in range(2):
                                    hs = slice(64 * h, 64 * h + 64)
                                    nc.tensor.matmul(
                                        sps[h][:, 0:T],
                                        Qsb[hs, :, p], Ksb[hs, :, p],
                                        start=(p == 0), stop=(p == P - 1),
                                        skip_group_check=True)
                            for h in range(2):
                                nmax = pbs.tile([128, 1], F32, tag=f"nm{h}")
                                nc.vector.reduce_max(
                                    nmax[:], sps[h][:, 0:T],
                                    axis=mybir.AxisListType.X, negate=True)
                                E = pbs.tile([128, T], BF16, tag=f"E{h}")
                                esum = pbs.tile([128, 1], F32, tag=f"es{h}")
                                nc.scalar.activation(E[:], sps[h][:, 0:T],
                                                     AF.Exp,
                                                     bias=nmax[:, 0:1],
                                                     accum_out=esum[:])
                                rinv = pbs.tile([128, 1], F32, tag=f"ri{h}")
                                nc.vector.reciprocal(rinv[:], esum[:])
                                En = pbs.tile([128, T], BF16, tag=f"En{h}")
                                nc.vector.tensor_scalar_mul(
                                    En[:], E[:], rinv[:, 0:1])
                                for qb in range(2):
                                    tps = pbt.tile([128, 1024], BF16,
                                                   tag="tps")
                                    nc.tensor.matmul(
                                        tps[:, 0:128],
                                        En[:, 128 * qb:128 * qb + 128],
                                        ident[:], is_transpose=True)
                                    nc.scalar.copy(
                                        STts[h][:, qb,
                                                128 * jb:128 * jb + 128],
                                        tps[:, 0:128])
                    pbq_cm.__exit__(None, None, None)
                    pbk_cm.__exit__(None, None, None)

                    # ---- phase C: O = softmax(scores) @ V ----
                    with tc.tile_pool(name=f"C{l}_{rep}", bufs=2) as pc, \
                         tc.tile_pool(name=f"Cs{l}_{rep}", bufs=4) as pcs, \
                         tc.tile_pool(name=f"Cp{l}_{rep}", bufs=1,
                                      space="PSUM") as pcp:
                        for h in range(2 if ph_on('C', l) else 0):
                            vts = []
                            for qb in range(2):
                                Vt = pc.tile([128, HID, P], BF16,
                                             tag=f"Vt{qb}")
                                for cc in range(8):
                                    eng = nc.scalar if (cc + qb) % 2 else \
                                        nc.sync
                                    eng.dma_start(
                                        Vt[:, 8 * cc:8 * cc + 8, :],
                                        Vd[64 * h + 8 * cc:
                                           64 * h + 8 * cc + 8,
                                           128 * qb:128 * qb + 128, :]
                                        .rearrange("c q p -> q c p"))
                                vts.append(Vt)
                            for jb in range(2):
                                for cpg in range(8):
                                    opss = [pcp.tile([128, 512], F32,
                                                     tag=f"ops{i}",
                                                     name=f"ops{i}")
                                            for i in range(4)]
                                    for qb in range(2):
                                        for i in range(4):
                                            cp = 4 * cpg + i
                                            nc.tensor.matmul(
                                                opss[i][:],
                                                STts[h][:, qb,
                                                        128 * jb:128 * jb + 128],
                                                vts[qb][:, 2 * cp:2 * cp + 2, :]
                                                .rearrange("q c p -> q (c p)"),
                                                start=(qb == 0),
                                                stop=(qb == 1))
                                    for i in range(4):
                                        cp = 4 * cpg + i
                                        ost = pcs.tile([128, 512], BF16,
                                                       tag="ost")
                                        nc.scalar.copy(ost[:], opss[i][:])
                                        nc.gpsimd.dma_start(
                                            Od[64 * h + 2 * cp:
                                               64 * h + 2 * cp + 2,
                                               128 * jb:128 * jb + 128, :]
                                            .rearrange("c j p -> j c p"),
                                            ost.rearrange("j (c p) -> j c p",
                                                          c=2))
                    paw_cm.__exit__(None, None, None)
                    pbst_cm.__exit__(None, None, None)

                    # ---- phase D: unify conv (col-tiled pairs) + RS ----
                    Ud = [dram.tile([4, HID, 16, P], BF16,
                                    tag=f"Ud{q}_{rep}", name=f"Ud{q}")
                          for q in range(4)]
                    RSo = [dram.tile([HID, 16, P], BF16,
                                     tag=f"RSo{q}_{rep}", name=f"RSo{q}")
                           for q in range(4)]
                    with tc.tile_pool(name=f"D{l}_{rep}", bufs=3) as pd, \
                         tc.tile_pool(name=f"Dw{l}_{rep}", bufs=1) as pdw, \
                         tc.tile_pool(name=f"Dp{l}_{rep}", bufs=3,
                                      space="PSUM") as pdp:
                        uw = pdw.tile([2 * HID, 9, HID], BF16, tag="uw")
                        if ph_on('D', l):
                            nc.sync.dma_start(uw[:], uwT_in.ap()[l])
                        for i in range(T // 4 if ph_on('D', l) else 0):
                            g0 = 4 * i
                            q, r = g0 // 64, (g0 % 64) // 16
                            s = g0 % 16
                            och = pd.tile([128, 4, 16, 16], BF16, tag="och")
                            nc.sync.dma_start(
                                och.rearrange("c a y x -> c (a y x)"),
                                Od[:, g0:g0 + 4, :]
                                .rearrange("c t p -> c (t p)"))
                            tsl = pd.tile([128, 2, P], BF16, tag="tsl")
                            for half in range(2):
                                nc.sync.dma_start(
                                    tsl[64 * half:64 * half + 64]
                                    .rearrange("c t p -> c (t p)"),
                                    Td[:, g0 + 2 * half:g0 + 2 * half + 2, :]
                                    .rearrange("c t p -> c (t p)"))
                            ups = pdp.tile([128, 2, 16, 16], F32, tag="ups")
                            for d, (dy, dx) in enumerate(OFFS):
                                liy, loy, cy = _rng(dy)
                                lix, lox, cx = _rng(dx)
                                for cg in range(2):
                                    nc.tensor.matmul(
                                        ups[64 * cg:64 * cg + 64, :,
                                            loy:loy + cy, lox:lox + cx],
                                        uw[:, d, :],
                                        och[:, 2 * cg:2 * cg + 2,
                                            liy:liy + cy, lix:lix + cx],
                                        start=(d == 0), stop=(d == 8),
                                        skip_group_check=True)
                            ust = pd.tile([128, 2 * P], BF16, tag="ust")
                            nc.vector.scalar_tensor_tensor(
                                ust[:], tsl.rearrange("c t p -> c (t p)"),
                                sel_sb[:, r:r + 1],
                                ups.rearrange("c a y x -> c (a y x)"),
                                op0=ALU.mult, op1=ALU.add)
                            for half in range(2):
                                nc.scalar.dma_start(
                                    Ud[q][r, :, s + 2 * half:s + 2 * half + 2,
                                          :].rearrange("c t p -> c (t p)"),
                                    ust[64 * half:64 * half + 64, :])
                            if g0 % 64 == 60 and ph_on('R', l):
                                nc.gpsimd.collective_compute(
                                    "ReduceScatter", ALU.add,
                                    replica_groups=GROUPS,
                                    ins=[Ud[q].opt()], outs=[RSo[q].opt()])

                    # ---- phase E: LN1 + MLP + LN2, two quarters/round ----
                    AGi = [dram.tile([HID, 16, P], BF16,
                                     tag=f"AGi{q}_{rep}", name=f"AGi{q}")
                           for q in range(4)]
                    AGo = [dram.tile([4, HID, 16, P], BF16,
                                     tag=f"AGo{q}_{rep}", name=f"AGo{q}")
                           for q in range(4)]
                    with tc.tile_pool(name=f"E{l}_{rep}", bufs=1) as pe, \
                         tc.tile_pool(name=f"Es{l}_{rep}", bufs=2) as pes, \
                         tc.tile_pool(name=f"Ew{l}_{rep}", bufs=1) as pew, \
                         tc.tile_pool(name=f"Ep{l}_{rep}", bufs=1,
                                      space="PSUM") as pep, \
                         tc.tile_pool(name=f"Ep2{l}_{rep}", bufs=2,
                                      space="PSUM") as pep2:
                        m1w = pew.tile([128, 2, 128], BF16, tag="m1w")
                        m1bs = pew.tile([2 * HID, 2], F32, tag="m1bs")
                        m2w = pew.tile([2 * HID, 2, HID], BF16, tag="m2w")
                        m2bs = pew.tile([128, 1], F32, tag="m2bs")
                        ln_w = pew.tile([128, 2, P], F32, tag="ln_w")
                        ln_b = pew.tile([128, 2, P], F32, tag="ln_b")
                        if ph_on('E', l):
                            nc.sync.dma_start(m1w[:], m1w2_in.ap()[l])
                            nc.sync.dma_start(m1bs[:], m1b_in.ap()[l])
                            nc.sync.dma_start(m2w[:], m2T_in.ap()[l])
                            nc.sync.dma_start(m2bs[:], m2b2_in.ap()[l])
                            for half in range(2):
                                hs = slice(64 * half, 64 * half + 64)
                                nc.sync.dma_start(
                                    ln_w[hs], lnw_in.ap()[l]
                                    .rearrange("w c p -> c w p"))
                                nc.sync.dma_start(
                                    ln_b[hs], lnb_in.ap()[l]
                                    .rearrange("w c p -> c w p"))

                        def layernorm(xin, wi, out_b, out_f=None):
                            xsq = pe.tile([128, 16, P], F32, tag="xsq")
                            nc.gpsimd.tensor_mul(xsq[:, 0:8], xin[:, 0:8],
                                                 xin[:, 0:8])
                            nc.vector.tensor_mul(xsq[:, 8:16], xin[:, 8:16],
                                                 xin[:, 8:16])
                            stats = pes.tile([128, 32], F32, tag="stats")
                            nc.vector.reduce_sum(stats[:, 0:16], xin[:],
                                                 axis=mybir.AxisListType.X)
                            nc.vector.reduce_sum(stats[:, 16:32], xsq[:],
                                                 axis=mybir.AxisListType.X)
                            sp = pep2.tile([128, 512], F32, tag="sp")
                            nc.tensor.matmul(sp[:, 0:32], onesblk[:],
                                             stats[:], start=True, stop=True)
                            mu = pes.tile([128, 48], F32, tag="mu")
                            nc.scalar.mul(mu[:, 0:16], sp[:, 0:16],
                                          1.0 / 16384)
                            nc.scalar.mul(mu[:, 16:32], sp[:, 16:32],
                                          1.0 / 16384)
                            nc.scalar.square(mu[:, 32:48], mu[:, 0:16])
                            var = pes.tile([128, 32], F32, tag="var")
                            nc.vector.tensor_sub(var[:, 0:16], mu[:, 16:32],
                                                 mu[:, 32:48])
                            nc.scalar.activation(var[:, 16:32], var[:, 0:16],
                                                 AF.Sqrt, bias=eps_sb[:, 0:1])
                            rstd = pes.tile([128, 16], F32, tag="rstd")
                            nc.vector.reciprocal(rstd[:], var[:, 16:32])
                            mu_bc = mu[:, 0:16].unsqueeze(-1) \
                                .broadcast_to((128, 16, P))
                            rs_bc = rstd[:, 0:16].unsqueeze(-1) \
                                .broadcast_to((128, 16, P))
                            w_bc = ln_w[:, wi, :].unsqueeze(1).broadcast_to(
                                (128, 16, P))
                            b_bc = ln_b[:, wi, :].unsqueeze(1).broadcast_to(
                                (128, 16, P))
                            xn = pe.tile([128, 16, P], F32, tag="xn")
                            SPL = 5
                            for eng, ts in ((nc.gpsimd, slice(0, SPL)),
                                            (nc.vector, slice(SPL, 16))):
                                n = ts.stop - ts.start
                                eng.tensor_sub(xn[:, ts], xin[:, ts],
                                               mu_bc[:, ts])
                                eng.tensor_mul(xn[:, ts], xn[:, ts],
                                               rs_bc[:, ts])
                                eng.tensor_mul(xn[:, ts], xn[:, ts],
                                               w_bc[:, ts])
                                if out_f is not None:
                                    eng.tensor_add(out_f[:, ts], xn[:, ts],
                                                   b_bc[:, ts])
                                else:
                                    eng.tensor_add(out_b[:, ts], xn[:, ts],
                                                   b_bc[:, ts])
                            if out_f is not None:
                                nc.vector.tensor_copy(out_b[:], out_f[:])

                        for rr in range(2 if ph_on('E', l) else 0):
                            xi = pe.tile([128, 16, P], BF16, tag="xi")
                            for half in range(2):
                                hs = slice(64 * half, 64 * half + 64)
                                nc.sync.dma_start(xi[hs], RSo[2 * rr + half])
                            idn_f = pe.tile([128, 16, P], F32, tag="idn_f")
                            idn_b = pe.tile([128, 16, P], BF16, tag="idn_b")
                            layernorm(xi, 0, idn_b, idn_f)
                            m1sb = [pe.tile([128, 2, 16, P], BF16,
                                            tag=f"m1sb{qq}",
                                            name=f"m1sb{qq}")
                                    for qq in range(2)]
                            for c2 in range(8):
                                cs = slice(2 * c2, 2 * c2 + 2)
                                mps = {}
                                for mb in range(2):
                                    for qq in range(2):
                                        qs = slice(64 * qq, 64 * qq + 64)
                                        mp = pep.tile([128, 512], F32,
                                                      tag=f"mp{qq}{mb}")
                                        nc.tensor.matmul(
                                            mp[:], m1w[qs, mb, :],
                                            idn_b[qs, cs, :]
                                            .rearrange("c t p -> c (t p)"),
                                            start=True, stop=True)
                                        mps[(qq, mb)] = mp
                                for mb in range(2):
                                    for qq in range(2):
                                        nc.scalar.activation(
                                            m1sb[qq][:, mb, cs, :]
                                            .rearrange("c t p -> c (t p)"),
                                            mps[(qq, mb)][:], AF.Relu,
                                            bias=m1bs[:, mb:mb + 1])
                            x2f = pe.tile([128, 16, P], F32, tag="x2f")
                            for c2 in range(8):
                                cs = slice(2 * c2, 2 * c2 + 2)
                                mp2 = pep2.tile([128, 512], F32, tag="mp2")
                                for mb in range(2):
                                    for qq in range(2):
                                        nc.tensor.matmul(
                                            mp2[64 * qq:64 * qq + 64, :],
                                            m2w[:, mb, :],
                                            m1sb[qq][:, mb, cs, :]
                                            .rearrange("c t p -> c (t p)"),
                                            start=(mb == 0), stop=(mb == 1))
                                nc.vector.scalar_tensor_tensor(
                                    x2f[:, cs, :]
                                    .rearrange("c t p -> c (t p)"),
                                    mp2[:], m2bs[:, 0:1],
                                    idn_f[:, cs, :]
                                    .rearrange("c t p -> c (t p)"),
                                    op0=ALU.add, op1=ALU.add)
                            yb = pe.tile([128, 16, P], BF16, tag="yb")
                            layernorm(x2f, 1, yb)
                            for half in range(2):
                                q = 2 * rr + half
                                hs = slice(64 * half, 64 * half + 64)
                                nc.sync.dma_start(AGi[q][:], yb[hs])
                                if l == L - 1:
                                    ybv = yb.rearrange(
                                        "c t (y x) -> c t y x", y=16)
                                    nc.sync.dma_start(
                                        StripI[:, q, 0],
                                        ybv[hs, :, 0, :].unsqueeze(1))
                                    nc.sync.dma_start(
                                        StripI[:, q, 1],
                                        ybv[hs, :, 15, :].unsqueeze(1))
                                if ph_on('R', l) and l < L - 1:
                                    nc.gpsimd.collective_compute(
                                        "AllGather", ALU.bypass,
                                        replica_groups=GROUPS,
                                        ins=[AGi[q].opt()],
                                        outs=[AGo[q].opt()])
                        if l == L - 1 and ph_on('R', l):
                            nc.gpsimd.collective_compute(
                                "AllGather", ALU.bypass,
                                replica_groups=GROUPS,
                                ins=[StripI.opt()], outs=[StripO.opt()])
                    AGi_last = AGi
                    # write gathered T back to Td (DRAM->DRAM)
                    for q in range(4 if (ph_on('R', l) and l < L - 1)
                                   else 0):
                        nc.sync.dma_start(
                            Td[:, 64 * q:64 * q + 64, :]
                            .rearrange("c (r s) p -> c r s p", r=4),
                            AGo[q].rearrange("r c s p -> c r s p"))

                # ---------- output head (rank-symmetric tile-row bands) ----
                with tc.tile_pool(name=f"H{rep}", bufs=2) as ph, \
                     tc.tile_pool(name=f"Hw{rep}", bufs=1) as phw, \
                     tc.tile_pool(name=f"Hp{rep}", bufs=2, space="PSUM") as php:
                    ow1 = phw.tile([128, 5, HID], BF16, tag="ow1")
                    ob1 = phw.tile([HID, 1], F32, tag="ob1")
                    ow2 = phw.tile([HID, 4], BF16, tag="ow2")
                    ob2 = phw.tile([4, 1], F32, tag="ob2")
                    if ph_on('H'):
                        nc.sync.dma_start(ow1[:], outw1b2_in.ap())
                        nc.sync.dma_start(ob1[:], outb1_in.ap())
                        nc.sync.dma_start(ow2[:], outw2T_in.ap())
                        nc.sync.dma_start(ob2[:], outb2_in.ap())
                    ssb = phw.tile([128, 4, 4, 2, 16, 16], BF16,
                                   tag="ssb")
                    if ph_on('H'):
                        for half in range(2):
                            hs = slice(64 * half, 64 * half + 64)
                            nc.sync.dma_start(
                                ssb[hs], StripO.rearrange(
                                    "r c q e t x -> c r q e t x"))
                    for q in range(4 if ph_on('H') else 0):
                        # own band's 16 mid rows are rank-local in AGi_last[q]
                        xi2 = ph.tile([128, 16, P], BF16, tag="xi2", bufs=2)
                        for half in range(2):
                            hs = slice(64 * half, 64 * half + 64)
                            nc.scalar.dma_start(xi2[hs], AGi_last[q][:])
                        img = ph.tile([128, 18, 16, 16], BF16, tag="img",
                                      bufs=1)
                        nc.gpsimd.memset(
                            img[:, 0, :, :].rearrange("c t x -> c (t x)"),
                            0.0)
                        nc.gpsimd.memset(
                            img[:, 17, :, :].rearrange("c t x -> c (t x)"),
                            0.0)
                        xi4 = xi2.rearrange("c t (y x) -> c t y x", y=16)
                        for y in range(16):
                            eng = nc.vector if y % 2 == 0 else nc.gpsimd
                            eng.tensor_copy(img[:, 1 + y, :, :],
                                            xi4[:, :, y, :])
                        for rr in range(4):
                            nc.vector.scalar_tensor_tensor(
                                img[:, 0, :, :], ssb[:, rr, q, 1],
                                sel_sb[:, 4 + rr:5 + rr],
                                img[:, 0, :, :], op0=ALU.mult, op1=ALU.add)
                            nc.vector.scalar_tensor_tensor(
                                img[:, 17, :, :], ssb[:, rr, q, 0],
                                sel_sb[:, 8 + rr:9 + rr],
                                img[:, 17, :, :], op0=ALU.mult, op1=ALU.add)
                        if q >= 1:
                            nc.vector.scalar_tensor_tensor(
                                img[:, 0, :, :], ssb[:, 3, q - 1, 1],
                                sel_sb[:, 12:13],
                                img[:, 0, :, :], op0=ALU.mult, op1=ALU.add)
                        if q <= 2:
                            nc.vector.scalar_tensor_tensor(
                                img[:, 17, :, :], ssb[:, 0, q + 1, 0],
                                sel_sb[:, 13:14],
                                img[:, 17, :, :], op0=ALU.mult, op1=ALU.add)
                        imgf = img.rearrange("c a t x -> c a (t x)")
                        for ch in range(8):
                            oc0 = php.tile([HID, 2, 256], F32, tag="oc0")
                            oc1 = php.tile([HID, 2, 256], F32, tag="oc1")
                            for k in range(5):
                                for ps, offl, rs in (
                                        (oc0, OFFA, slice(0, 64)),
                                        (oc1, OFFB, slice(64, 128))):
                                    dy, dx = offl[k]
                                    lix, lox, cx = _rng(dx, 256)
                                    nc.tensor.matmul(
                                        ps[:, 0:2, lox:lox + cx],
                                        ow1[rs, k, :],
                                        imgf[rs, 2 * ch + dy:2 * ch + dy + 2,
                                             lix:lix + cx],
                                        start=(k == 0), stop=(k == 4),
                                        skip_group_check=True)
                            o1 = ph.tile([HID, 2 * 256], BF16, tag="o1")
                            nc.scalar.copy(
                                o1[:], oc0.rearrange("c a x -> c (a x)"))
                            nc.vector.tensor_add(
                                o1[:], o1[:],
                                oc1.rearrange("c a x -> c (a x)"))
                            o1r = ph.tile([HID, 2 * 256], BF16, tag="o1r")
                            nc.scalar.activation(o1r[:], o1[:], AF.Relu,
                                                 bias=ob1[:, 0:1])
                            p2 = php.tile([4, 2 * 256], F32, tag="p2")
                            nc.tensor.matmul(p2[:], ow2[:], o1r[:],
                                             start=True, stop=True)
                            ysb = ph.tile([4, 2 * 256], F32, tag="ysb")
                            nc.vector.tensor_scalar_add(ysb[:], p2[:],
                                                        ob2[:, 0:1])
                            nc.sync.dma_start(
                                y_out.ap()[:, q, 2 * ch:2 * ch + 2, :]
                                .rearrange("c t x -> c (t x)"), ysb[:])
    nc.finalize()
    return nc, taps


# ======================= host side =======================

def _prep_core_inputs(core, inputs):
    r = core % 4
    b = core // 4
    f32 = np.float32
    bf = lambda a: np.ascontiguousarray(np.asarray(a, f32)).astype(NBF)
    x = np.asarray(inputs["x"], f32)
    xtc = x[b].reshape(3, 16, 16, 16, 16).transpose(0, 1, 3, 2, 4) \
        .reshape(3, T, P)
    pos = np.asarray(inputs["pos"], f32).reshape(T, HID).T.copy()
    hsl = slice(2 * r * HID, 2 * r * HID + 2 * HID)

    offa_i = [dy * 3 + dx for dy, dx in OFFA]
    offb_i = [dy * 3 + dx for dy, dx in OFFB]
    qkv2 = np.empty((L, 128, 3, 5, 128), f32)
    for l in range(L):
        for i, (nm, sc) in enumerate((("qw", 0.25), ("kw", 0.25),
                                      ("vw", 1.0))):
            w = np.asarray(inputs[nm], f32)[l, hsl] * sc  # [128,64,3,3]
            wt = w.transpose(1, 2, 3, 0).reshape(HID, 9, 2 * HID)
            for k in range(5):
                sca = 0.5 if k == 0 else 1.0
                qkv2[l, 0:64, i, k, :] = wt[:, offa_i[k], :] * sca
                qkv2[l, 64:128, i, k, :] = wt[:, offb_i[k], :] * sca
    offs_i = [dy * 3 + dx for dy, dx in OFFS]
    uwp = np.empty((L, 2 * HID, 9, HID), f32)
    for l in range(L):
        w = np.asarray(inputs["uw"], f32)[l][:, hsl]      # [64,128,3,3]
        wt = w.transpose(1, 2, 3, 0).reshape(2 * HID, 9, HID)
        uwp[l] = wt[:, offs_i, :]
    m1 = np.asarray(inputs["mlp_w1"], f32)[:, :, :, 0, 0]  # [L,256,64]
    m1T = m1.transpose(0, 2, 1).reshape(L, HID, 2, 128)    # [L,64,2,128]
    m1w2 = np.concatenate([m1T, m1T], axis=1)              # [L,128,2,128]
    m1bp = np.asarray(inputs["mlp_b1"], f32).reshape(L, 2, 2 * HID) \
        .transpose(0, 2, 1).copy()                         # [L,128,2]
    m2 = np.asarray(inputs["mlp_w2"], f32)[:, :, :, 0, 0]  # [L,64,256]
    m2Tp = m2.transpose(0, 2, 1).reshape(L, 2, 2 * HID, HID) \
        .transpose(0, 2, 1, 3).copy()                      # [L,128,2,64]
    m2b = np.asarray(inputs["mlp_b2"], f32)                # [L,64]
    m2b2 = np.concatenate([m2b, m2b], axis=1).reshape(L, 128, 1)
    lnwp = np.stack([np.asarray(inputs["ln1_w"], f32).reshape(L, HID, P),
                     np.asarray(inputs["ln2_w"], f32).reshape(L, HID, P)], 1)
    lnbp = np.stack([np.asarray(inputs["ln1_b"], f32).reshape(L, HID, P),
                     np.asarray(inputs["ln2_b"], f32).reshape(L, HID, P)], 1)
    ow1 = np.asarray(inputs["out_w1"], f32)                # [64,64,3,3]
    ow1T = ow1.transpose(1, 2, 3, 0).reshape(HID, 9, HID)
    ow1b2 = np.empty((128, 5, HID), f32)
    for k in range(5):
        sca = 0.5 if k == 0 else 1.0
        ow1b2[0:64, k, :] = ow1T[:, offa_i[k], :] * sca
        ow1b2[64:128, k, :] = ow1T[:, offb_i[k], :] * sca
    ow2 = np.asarray(inputs["out_w2"], f32)[:, :, 0, 0]    # [3,64]
    ow2T = np.zeros((HID, 4), f32)
    ow2T[:, :3] = ow2.T
    ob2 = np.zeros((4, 1), f32)
    ob2[:3, 0] = np.asarray(inputs["out_b2"], f32)
    sel = np.zeros((128, 14), f32)
    sel[:, r] = 1.0
    if r >= 1:
        sel[:, 4 + r - 1] = 1.0          # selprev
    if r <= 2:
        sel[:, 8 + r + 1] = 1.0          # selnext
    if r == 0:
        sel[:, 12] = 1.0                 # wrap from prev quarter (rank 3)
    if r == 3:
        sel[:, 13] = 1.0                 # wrap into next quarter (rank 0)
    onesblk = np.zeros((128, 128), f32)
    onesblk[0:64, 0:64] = 1.0
    onesblk[64:128, 64:128] = 1.0

    return {
        "xt": bf(xtc), "pos_in": pos.astype(np.float32),
        "semwT": bf(np.asarray(inputs["sem_w"], f32)[:, :, 0, 0].T),
        "semb": np.asarray(inputs["sem_b"], f32).reshape(HID, 1).copy(),
        "qkvw2": bf(qkv2), "uwT": bf(uwp), "m1w2": bf(m1w2), "m1b": m1bp,
        "m2T": bf(m2Tp), "m2b2": m2b2, "lnw": lnwp, "lnb": lnbp,
        "outw1b2": bf(ow1b2),
        "outb1": np.asarray(inputs["out_b1"], f32).reshape(HID, 1).copy(),
        "outw2T": bf(ow2T), "outb2": ob2,
        "ident_in": np.eye(128, dtype=NBF), "sel_in": sel,
        "onesblk_in": onesblk,
    }


def assemble_output(results):
    img = np.zeros((B, 3, 256, 256), np.float32)
    for c in range(N_CORES):
        b, r = c // 4, c % 4
        y = np.asarray(results[c]["y_out"], np.float32)  # [4,4,16,256]
        for q in range(4):
            rb = 16 * (4 * q + r)
            img[b, :, rb:rb + 16, :] = y[:3, q]
    return img


_CACHE = {}


def get_built(debug_taps=()):
    key = tuple(sorted(debug_taps))
    if key not in _CACHE:
        t0 = time.time()
        nc, taps = build(debug_taps)
        _CACHE[key] = (nc, taps)
        print(f"[kernel] build {time.time() - t0:.1f}s", file=sys.stderr)
    return _CACHE[key]


def run_cores(inputs, debug_taps=()):
    from concourse import bass2jax
    nc, taps = get_built(debug_taps)
    in_maps = [_prep_core_inputs(c, inputs) for c in range(N_CORES)]
    t0 = time.time()
    results = bass2jax.run_bass_via_pjrt(nc, in_maps, n_cores=N_CORES)
    print(f"[kernel] run {time.time() - t0:.1f}s", file=sys.stderr)
    return results


def kernel(**inputs):
    results = run_cores(inputs)
    return assemble_output(results)


# revision 26
# speedup vs baseline: 2.3457x; 1.2483x over previous
"""nn_CNNTransformer Trainium2 kernel — full-input/full-output contract.

Sharding (8 NeuronCores): 2 batch groups x 4 cores.  Within a group each
core computes the QKV convs + attention + partial unify conv for its 2 of
the 8 heads; unify partials (with the residual folded in on exactly one
rank via a one-hot multiplier) are ReduceScattered (bf16) over the group
in 4 tile-quarter chunks, LayerNorm+MLP+LayerNorm run on the core's tile
shard (two quarters at a time across the 128 partitions), and chunked
AllGathers rebuild the full activation for the next layer.

v2: all K=64 contractions are row-tiled across the two 64-partition
halves of the PE array (two concurrent matmuls in different row groups,
two PSUM banks, DVE add at eviction) and 64-col outputs are col-tiled —
this keeps the PE at the warm 2.4 GHz clock and roughly doubles matmul
throughput.  Activations live in DRAM as bf16.

Self-contained: only needs the concourse tree at /opt/trn_rl_repo.
"""
import os
import sys
import time

for _p in ("/opt/trn_rl_repo", "/root/.axon_site/_ro/trn_rl_repo"):
    if os.path.isdir(_p) and _p not in sys.path:
        sys.path.insert(0, _p)
        break

import numpy as np
import ml_dtypes

import concourse.bacc as bacc
import concourse.mybir as mybir
import concourse.tile as tile
B, NT, HID, HEADS, L = 2, 16, 64, 8, 2
T = NT * NT            # 256 tiles
P = 256                # pixels per 16x16 tile
N_CORES = 8
GROUPS = [[0, 1, 2, 3], [4, 5, 6, 7]]
BF16 = mybir.dt.bfloat16
F32 = mybir.dt.float32
NBF = ml_dtypes.bfloat16
AF = mybir.ActivationFunctionType
ALU = mybir.AluOpType

# 3x3 offsets, center first (full-region matmul opens each PSUM bank).
OFFS = [(1, 1)] + [(dy, dx) for dy in range(3) for dx in range(3)
                   if (dy, dx) != (1, 1)]
# bank0 gets half-center + OFFS[1,3,5,7]; bank1 half-center + OFFS[2,4,6,8]
OFFA = [OFFS[i] for i in (0, 1, 3, 5, 7)]
OFFB = [OFFS[i] for i in (0, 2, 4, 6, 8)]


def _rng(o, n=16):
    return max(o - 1, 0), max(1 - o, 0), n - abs(o - 1)


def build(debug_taps=(), only=None, reps=1):
    def ph_on(p, l=None):
        if only is None:
            return True
        return p in only or (l is not None and f"{p}{l}" in only)
    nc = bacc.Bacc(None, target_bir_lowering=False, debug=False)

    xt = nc.dram_tensor("xt", [3, T, P], BF16, kind="ExternalInput")
    pos_in = nc.dram_tensor("pos_in", [HID, T], F32, kind="ExternalInput")
    semwT = nc.dram_tensor("semwT", [3, HID], BF16, kind="ExternalInput")
    semb = nc.dram_tensor("semb", [HID, 1], F32, kind="ExternalInput")
    qkvw2_in = nc.dram_tensor("qkvw2", [L, 128, 3, 5, 128], BF16,
                              kind="ExternalInput")
    uwT_in = nc.dram_tensor("uwT", [L, 2 * HID, 9, HID], BF16,
                            kind="ExternalInput")
    m1w2_in = nc.dram_tensor("m1w2", [L, 128, 2, 128], BF16,
                             kind="ExternalInput")
    m1b_in = nc.dram_tensor("m1b", [L, 2 * HID, 2], F32,
                            kind="ExternalInput")
    m2T_in = nc.dram_tensor("m2T", [L, 2 * HID, 2, HID], BF16,
                            kind="ExternalInput")
    m2b2_in = nc.dram_tensor("m2b2", [L, 128, 1], F32, kind="ExternalInput")
    lnw_in = nc.dram_tensor("lnw", [L, 2, HID, P], F32, kind="ExternalInput")
    lnb_in = nc.dram_tensor("lnb", [L, 2, HID, P], F32, kind="ExternalInput")
    outw1b2_in = nc.dram_tensor("outw1b2", [128, 5, HID], BF16,
                                kind="ExternalInput")
    outb1_in = nc.dram_tensor("outb1", [HID, 1], F32, kind="ExternalInput")
    outw2T_in = nc.dram_tensor("outw2T", [HID, 4], BF16,
                               kind="ExternalInput")
    outb2_in = nc.dram_tensor("outb2", [4, 1], F32, kind="ExternalInput")
    ident_in = nc.dram_tensor("ident_in", [128, 128], BF16,
                              kind="ExternalInput")
    sel_in = nc.dram_tensor("sel_in", [128, 14], F32, kind="ExternalInput")
    onesblk_in = nc.dram_tensor("onesblk_in", [128, 128], F32,
                                kind="ExternalInput")
    y_out = nc.dram_tensor("y_out", [4, 4, 16, NT * 16], F32,
                           kind="ExternalOutput")

    taps = {}

    def tap(name, shape, dtype=BF16):
        if name in debug_taps and name not in taps:
            taps[name] = nc.dram_tensor("tap_" + name, shape, dtype,
                                        kind="ExternalOutput")
        return taps.get(name)

    with tile.TileContext(nc) as tc:
        with tc.tile_pool(name="dram", bufs=1, space="DRAM") as dram, \
             tc.tile_pool(name="persist", bufs=1) as persist:

            ident = persist.tile([128, 128], BF16, tag="ident")
            onesblk = persist.tile([128, 128], F32, tag="onesblk")
            sel_sb = persist.tile([128, 14], F32, tag="sel_sb")
            eps_sb = persist.tile([128, 1], F32, tag="eps_sb")
            nc.sync.dma_start(ident[:], ident_in.ap())
            nc.sync.dma_start(onesblk[:], onesblk_in.ap())
            nc.sync.dma_start(sel_sb[:], sel_in.ap())
            nc.gpsimd.memset(eps_sb[:], 1e-5)

            for rep in range(reps):
                Qd = dram.tile([128, T, P], BF16, tag=f"Qd{rep}", name="Qd")
                Vd = dram.tile([128, T, P], BF16, tag=f"Vd{rep}", name="Vd")
                Od = dram.tile([128, T, P], BF16, tag=f"Od{rep}", name="Od")
                Td = dram.tile([HID, T, P], BF16, tag=f"Td{rep}", name="Td")
                StripI = dram.tile([HID, 4, 2, 16, 16], BF16,
                                   tag=f"StripI{rep}", name="StripI")
                StripO = dram.tile([4, HID, 4, 2, 16, 16], BF16,
                                   tag=f"StripO{rep}", name="StripO")
                # ---------- stage 0: sem 1x1 conv + ReLU + pos -> Td -------
                with tc.tile_pool(name=f"s0{rep}", bufs=3) as s0, \
                     tc.tile_pool(name=f"s0w{rep}", bufs=1) as s0w, \
                     tc.tile_pool(name=f"s0p{rep}", bufs=2, space="PSUM") as s0p:
                    swt = s0w.tile([128, HID], BF16, tag="swt")
                    sbt = s0w.tile([HID, 1], F32, tag="sbt")
                    posf_sb = s0w.tile([HID, T], F32, tag="posf_sb")
                    nc.sync.dma_start(posf_sb[:], pos_in.ap())
                    if ph_on('S'):
                        nc.sync.dma_start(swt[0:3, :], semwT.ap())
                        nc.sync.dma_start(swt[64:67, :], semwT.ap())
                        nc.sync.dma_start(sbt[:], semb.ap())
                    for i in range(T // 8 if ph_on('S') else 0):
                        g0 = 8 * i
                        xs = s0.tile([128, 4, P], BF16, tag="xs")
                        nc.sync.dma_start(
                            xs[0:3, :, :].rearrange("c t p -> c (t p)"),
                            xt.ap()[:, g0:g0 + 4, :]
                            .rearrange("c t p -> c (t p)"))
                        nc.sync.dma_start(
                            xs[64:67, :, :].rearrange("c t p -> c (t p)"),
                            xt.ap()[:, g0 + 4:g0 + 8, :]
                            .rearrange("c t p -> c (t p)"))
                        pss = []
                        for u in range(4):
                            half = u // 2
                            hs = slice(64 * half, 64 * half + 3)
                            ts = slice(2 * (u % 2), 2 * (u % 2) + 2)
                            ps = s0p.tile([HID, 2 * P], F32, tag=f"ps{u}",
                                          name=f"ps{u}")
                            nc.tensor.matmul(
                                ps[:], swt[hs, :],
                                xs[hs, ts, :].rearrange("c t p -> c (t p)"),
                                start=True, stop=True)
                            pss.append(ps)
                        for u, ps in enumerate(pss):
                            g = g0 + 2 * u
                            tch = s0.tile([HID, 2, P], F32, tag=f"tch{u}",
                                          name="tch")
                            nc.scalar.activation(
                                tch.rearrange("c t p -> c (t p)"), ps[:],
                                AF.Relu, bias=sbt[:, 0:1])
                            ob = s0.tile([HID, 2, P], BF16, tag=f"ob{u}",
                                         name="ob")
                            eng = nc.vector if u % 2 == 0 else nc.gpsimd
                            eng.tensor_add(
                                ob[:], tch[:],
                                posf_sb[:, g:g + 2].unsqueeze(-1)
                                .broadcast_to((HID, 2, P)))
                            nc.sync.dma_start(
                                Td[:, g:g + 2, :]
                                .rearrange("c t p -> c (t p)"),
                                ob.rearrange("c t p -> c (t p)"))

                # ---------- transformer layers ----------
                for l in range(L):
                    # ---- phase A: QKV convs, row-tiled halves ----
                    pbst_cm = tc.tile_pool(name=f"Bst{l}_{rep}", bufs=1)
                    pbst = pbst_cm.__enter__()
                    STts = [pbst.tile([128, 2, T], BF16, tag=f"STt{h}",
                                      name=f"STt{h}") for h in range(2)]
                    paw_cm = tc.tile_pool(name=f"Aw{l}_{rep}", bufs=1)
                    paw = paw_cm.__enter__()
                    qkvw = paw.tile([128, 3, 5, 128], BF16, tag="qkvw")
                    if ph_on('A', l) or ph_on('B', l):
                        nc.sync.dma_start(qkvw[:], qkvw2_in.ap()[l])
                    pbk_cm = tc.tile_pool(name=f"BK{l}_{rep}", bufs=1)
                    pbk = pbk_cm.__enter__()
                    Ksb = pbk.tile([128, T, P], BF16, tag="Ksb")

                    stg = {}

                    def conv_pair(tp, ci, pool, pspool, dstd, S=None):
                        """3x3 conv for tile-pair tp of conv ci, row-tiled."""
                        ts2 = slice(2 * tp, 2 * tp + 2)
                        if S is None:
                            S = pool.tile([128, 2, 16, 16], BF16,
                                          tag=f"S{ci}", name="S")
                            for half in range(2):
                                nc.sync.dma_start(
                                    S[64 * half:64 * half + 64]
                                    .rearrange("c a y x -> c (a y x)"),
                                    Td[:, ts2, :]
                                    .rearrange("c t p -> c (t p)"))
                        ps0 = pspool.tile([128, 2, 16, 16], F32,
                                          tag=f"cps{ci}0", name="cps0")
                        ps1 = pspool.tile([128, 2, 16, 16], F32,
                                          tag=f"cps{ci}1", name="cps1")
                        for k in range(5):
                            for ps, offl, rs in (
                                    (ps0, OFFA, slice(0, 64)),
                                    (ps1, OFFB, slice(64, 128))):
                                dy, dx = offl[k]
                                liy, loy, cy = _rng(dy)
                                lix, lox, cx = _rng(dx)
                                nc.tensor.matmul(
                                    ps[:, :, loy:loy + cy, lox:lox + cx],
                                    qkvw[rs, ci, k, :],
                                    S[rs, :, liy:liy + cy, lix:lix + cx],
                                    start=(k == 0), stop=(k == 4),
                                    skip_group_check=True)
                        if ci == 2:
                            st = stg["p"].tile([128, 512], BF16,
                                               tag="s2", name="st")
                            nc.scalar.copy(
                                st[:], ps0.rearrange("c a y x -> c (a y x)"))
                            nc.vector.tensor_add(
                                st[:], st[:],
                                ps1.rearrange("c a y x -> c (a y x)"))
                            nc.gpsimd.dma_start(
                                dstd[:, ts2, :].rearrange("c t p -> c (t p)"),
                                st[:])
                            return
                        t0 = stg["p"].tile([128, 512], F32, tag=f"t{ci}",
                                           name="t0")
                        nc.scalar.copy(
                            t0[:], ps0.rearrange("c a y x -> c (a y x)"))
                        if dstd is None:
                            nc.vector.tensor_add(
                                Ksb[:, ts2, :].rearrange("c t p -> c (t p)"),
                                t0[:], ps1.rearrange("c a y x -> c (a y x)"))
                        else:
                            st = stg["p"].tile([128, 512], BF16,
                                               tag="s0", name="st")
                            nc.vector.tensor_add(
                                st[:], t0[:],
                                ps1.rearrange("c a y x -> c (a y x)"))
                            nc.gpsimd.dma_start(
                                dstd[:, ts2, :].rearrange("c t p -> c (t p)"),
                                st[:])

                    pbq_cm = tc.tile_pool(name=f"BQ{l}_{rep}", bufs=1)
                    pbq = pbq_cm.__enter__()
                    Qsb = pbq.tile([128, 128, P], BF16, tag="Qsb")
                    with tc.tile_pool(name=f"A{l}_{rep}", bufs=3) as pa, \
                         tc.tile_pool(name=f"As{l}_{rep}", bufs=1) as pas, \
                         tc.tile_pool(name=f"Ap{l}_{rep}", bufs=1,
                                      space="PSUM") as pap:
                        stg["p"] = pas
                        for tp in range(T // 2 if ph_on('A', l) else 0):
                            ts2 = slice(2 * tp, 2 * tp + 2)
                            S = pa.tile([128, 2, 16, 16], BF16, tag="S",
                                        name="S")
                            for half in range(2):
                                nc.sync.dma_start(
                                    S[64 * half:64 * half + 64]
                                    .rearrange("c a y x -> c (a y x)"),
                                    Td[:, ts2, :]
                                    .rearrange("c t p -> c (t p)"))
                            conv_pair(tp, 0, pa, pap, Qd, S=S)
                            conv_pair(tp, 1, pa, pap, None, S=S)
                            conv_pair(tp, 2, pa, pap, Vd, S=S)
                            if tp == 67 and ph_on('B', l):
                                nc.sync.dma_start(
                                    Qsb[:], Qd[:, 0:128, :])

                    # ---- phase B: scores + softmax (both heads row-tiled) --
                    with tc.tile_pool(name=f"Bs{l}_{rep}", bufs=2) as pbs, \
                         tc.tile_pool(name=f"Bp{l}_{rep}", bufs=2,
                                      space="PSUM") as pbp, \
                         tc.tile_pool(name=f"Bt{l}_{rep}", bufs=2,
                                      space="PSUM") as pbt:
                        for jb in range(2 if ph_on('B', l) else 0):
                            if jb == 1:
                                nc.sync.dma_start(
                                    Qsb[:],
                                    Qd[:, 128 * jb:128 * jb + 128, :])
                            sps = [pbp.tile([128, 512], F32, tag=f"sps{h}",
                                             name=f"sps{h}")
                                   for h in range(2)]
                            for p in range(P):
                                for h in range(2):
                                    hs = slice(64 * h, 64 * h + 64)
                                    nc.tensor.matmul(
                                        sps[h][:, 0:T],
                                        Qsb[hs, :, p], Ksb[hs, :, p],
                                        start=(p == 0), stop=(p == P - 1),
                                        skip_group_check=True)
                            for h in range(2):
                                nmax = pbs.tile([128, 1], F32, tag=f"nm{h}")
                                nc.vector.reduce_max(
                                    nmax[:], sps[h][:, 0:T],
                                    axis=mybir.AxisListType.X, negate=True)
                                E = pbs.tile([128, T], BF16, tag=f"E{h}")
                                esum = pbs.tile([128, 1], F32, tag=f"es{h}")
                                nc.scalar.activation(E[:], sps[h][:, 0:T],
                                                     AF.Exp,
                                                     bias=nmax[:, 0:1],
                                                     accum_out=esum[:])
                                rinv = pbs.tile([128, 1], F32, tag=f"ri{h}")
                                nc.vector.reciprocal(rinv[:], esum[:])
                                En = pbs.tile([128, T], BF16, tag=f"En{h}")
                                nc.vector.tensor_scalar_mul(
                                    En[:], E[:], rinv[:, 0:1])
                                for qb in range(2):
                                    tps = pbt.tile([128, 1024], BF16,
                                                   tag="tps")
                                    nc.tensor.matmul(
                                        tps[:, 0:128],
                                        En[:, 128 * qb:128 * qb + 128],
                                        ident[:], is_transpose=True)
                                    nc.scalar.copy(
                                        STts[h][:, qb,
                                                128 * jb:128 * jb + 128],
                                        tps[:, 0:128])
                    pbq_cm.__exit__(None, None, None)
                    pbk_cm.__exit__(None, None, None)

                    # ---- phase C: O = softmax(scores) @ V ----
                    with tc.tile_pool(name=f"C{l}_{rep}", bufs=2) as pc, \
                         tc.tile_pool(name=f"Cs{l}_{rep}", bufs=4) as pcs, \
                         tc.tile_pool(name=f"Cp{l}_{rep}", bufs=1,
                                      space="PSUM") as pcp:
                        for h in range(2 if ph_on('C', l) else 0):
                            vts = []
                            for qb in range(2):
                                Vt = pc.tile([128, HID, P], BF16,
                                             tag=f"Vt{qb}")
                                for cc in range(8):
                                    eng = (nc.scalar, nc.sync,
                                           nc.gpsimd)[(cc + qb) % 3]
                                    eng.dma_start(
                                        Vt[:, 8 * cc:8 * cc + 8, :],
                                        Vd[64 * h + 8 * cc:
                                           64 * h + 8 * cc + 8,
                                           128 * qb:128 * qb + 128, :]
                                        .rearrange("c q p -> q c p"))
                                vts.append(Vt)
                            for jb in range(2):
                                for cpg in range(8):
                                    opss = [pcp.tile([128, 512], F32,
                                                     tag=f"ops{i}",
                                                     name=f"ops{i}")
                                            for i in range(4)]
                                    for qb in range(2):
                                        for i in range(4):
                                            cp = 4 * cpg + i
                                            nc.tensor.matmul(
                                                opss[i][:],
                                                STts[h][:, qb,
                                                        128 * jb:128 * jb + 128],
                                                vts[qb][:, 2 * cp:2 * cp + 2, :]
                                                .rearrange("q c p -> q (c p)"),
                                                start=(qb == 0),
                                                stop=(qb == 1))
                                    for i in range(4):
                                        cp = 4 * cpg + i
                                        ost = pcs.tile([128, 512], BF16,
                                                       tag="ost")
                                        nc.scalar.copy(ost[:], opss[i][:])
                                        nc.gpsimd.dma_start(
                                            Od[64 * h + 2 * cp:
                                               64 * h + 2 * cp + 2,
                                               128 * jb:128 * jb + 128, :]
                                            .rearrange("c j p -> j c p"),
                                            ost.rearrange("j (c p) -> j c p",
                                                          c=2))
                    paw_cm.__exit__(None, None, None)
                    pbst_cm.__exit__(None, None, None)

                    # ---- phase D: unify conv (col-tiled pairs) + RS ----
                    Ud = [dram.tile([4, HID, 16, P], BF16,
                                    tag=f"Ud{q}_{rep}", name=f"Ud{q}")
                          for q in range(4)]
                    RSo = [dram.tile([HID, 16, P], BF16,
                                     tag=f"RSo{q}_{rep}", name=f"RSo{q}")
                           for q in range(4)]
                    with tc.tile_pool(name=f"D{l}_{rep}", bufs=3) as pd, \
                         tc.tile_pool(name=f"Dw{l}_{rep}", bufs=1) as pdw, \
                         tc.tile_pool(name=f"Dp{l}_{rep}", bufs=3,
                                      space="PSUM") as pdp:
                        uw = pdw.tile([2 * HID, 9, HID], BF16, tag="uw")
                        if ph_on('D', l):
                            nc.sync.dma_start(uw[:], uwT_in.ap()[l])
                        for i in range(T // 4 if ph_on('D', l) else 0):
                            g0 = 4 * i
                            q, r = g0 // 64, (g0 % 64) // 16
                            s = g0 % 16
                            och = pd.tile([128, 4, 16, 16], BF16, tag="och")
                            nc.gpsimd.dma_start(
                                och.rearrange("c a y x -> c (a y x)"),
                                Od[:, g0:g0 + 4, :]
                                .rearrange("c t p -> c (t p)"))
                            tsl = pd.tile([128, 2, P], BF16, tag="tsl")
                            for half in range(2):
                                nc.sync.dma_start(
                                    tsl[64 * half:64 * half + 64]
                                    .rearrange("c t p -> c (t p)"),
                                    Td[:, g0 + 2 * half:g0 + 2 * half + 2, :]
                                    .rearrange("c t p -> c (t p)"))
                            ups = pdp.tile([128, 2, 16, 16], F32, tag="ups")
                            for d, (dy, dx) in enumerate(OFFS):
                                liy, loy, cy = _rng(dy)
                                lix, lox, cx = _rng(dx)
                                for cg in range(2):
                                    nc.tensor.matmul(
                                        ups[64 * cg:64 * cg + 64, :,
                                            loy:loy + cy, lox:lox + cx],
                                        uw[:, d, :],
                                        och[:, 2 * cg:2 * cg + 2,
                                            liy:liy + cy, lix:lix + cx],
                                        start=(d == 0), stop=(d == 8),
                                        skip_group_check=True)
                            ust = pd.tile([128, 2 * P], BF16, tag="ust")
                            nc.vector.scalar_tensor_tensor(
                                ust[:], tsl.rearrange("c t p -> c (t p)"),
                                sel_sb[:, r:r + 1],
                                ups.rearrange("c a y x -> c (a y x)"),
                                op0=ALU.mult, op1=ALU.add)
                            for half in range(2):
                                nc.scalar.dma_start(
                                    Ud[q][r, :, s + 2 * half:s + 2 * half + 2,
                                          :].rearrange("c t p -> c (t p)"),
                                    ust[64 * half:64 * half + 64, :])
                            if g0 % 64 == 60 and ph_on('R', l):
                                nc.gpsimd.collective_compute(
                                    "ReduceScatter", ALU.add,
                                    replica_groups=GROUPS,
                                    ins=[Ud[q].opt()], outs=[RSo[q].opt()])

                    # ---- phase E: LN1 + MLP + LN2, two quarters/round ----
                    AGi = [dram.tile([HID, 16, P], BF16,
                                     tag=f"AGi{q}_{rep}", name=f"AGi{q}")
                           for q in range(4)]
                    AGo = [dram.tile([4, HID, 16, P], BF16,
                                     tag=f"AGo{q}_{rep}", name=f"AGo{q}")
                           for q in range(4)]
                    with tc.tile_pool(name=f"E{l}_{rep}", bufs=1) as pe, \
                         tc.tile_pool(name=f"Es{l}_{rep}", bufs=2) as pes, \
                         tc.tile_pool(name=f"Ew{l}_{rep}", bufs=1) as pew, \
                         tc.tile_pool(name=f"Ep{l}_{rep}", bufs=1,
                                      space="PSUM") as pep, \
                         tc.tile_pool(name=f"Ep2{l}_{rep}", bufs=2,
                                      space="PSUM") as pep2:
                        m1w = pew.tile([128, 2, 128], BF16, tag="m1w")
                        m1bs = pew.tile([2 * HID, 2], F32, tag="m1bs")
                        m2w = pew.tile([2 * HID, 2, HID], BF16, tag="m2w")
                        m2bs = pew.tile([128, 1], F32, tag="m2bs")
                        ln_w = pew.tile([128, 2, P], F32, tag="ln_w")
                        ln_b = pew.tile([128, 2, P], F32, tag="ln_b")
                        if ph_on('E', l):
                            nc.sync.dma_start(m1w[:], m1w2_in.ap()[l])
                            nc.sync.dma_start(m1bs[:], m1b_in.ap()[l])
                            nc.sync.dma_start(m2w[:], m2T_in.ap()[l])
                            nc.sync.dma_start(m2bs[:], m2b2_in.ap()[l])
                            for half in range(2):
                                hs = slice(64 * half, 64 * half + 64)
                                nc.sync.dma_start(
                                    ln_w[hs], lnw_in.ap()[l]
                                    .rearrange("w c p -> c w p"))
                                nc.sync.dma_start(
                                    ln_b[hs], lnb_in.ap()[l]
                                    .rearrange("w c p -> c w p"))

                        def layernorm(xin, wi, out_b, out_f=None):
                            xsq = pe.tile([128, 16, P], F32, tag="xsq")
                            nc.gpsimd.tensor_mul(xsq[:, 0:8], xin[:, 0:8],
                                                 xin[:, 0:8])
                            nc.vector.tensor_mul(xsq[:, 8:16], xin[:, 8:16],
                                                 xin[:, 8:16])
                            stats = pes.tile([128, 32], F32, tag="stats")
                            nc.vector.reduce_sum(stats[:, 0:16], xin[:],
                                                 axis=mybir.AxisListType.X)
                            nc.vector.reduce_sum(stats[:, 16:32], xsq[:],
                                                 axis=mybir.AxisListType.X)
                            sp = pep2.tile([128, 512], F32, tag="sp")
                            nc.tensor.matmul(sp[:, 0:32], onesblk[:],
                                             stats[:], start=True, stop=True)
                            mu = pes.tile([128, 48], F32, tag="mu")
                            nc.scalar.mul(mu[:, 0:16], sp[:, 0:16],
                                          1.0 / 16384)
                            nc.scalar.mul(mu[:, 16:32], sp[:, 16:32],
                                          1.0 / 16384)
                            nc.scalar.square(mu[:, 32:48], mu[:, 0:16])
                            var = pes.tile([128, 32], F32, tag="var")
                            nc.vector.tensor_sub(var[:, 0:16], mu[:, 16:32],
                                                 mu[:, 32:48])
                            nc.scalar.activation(var[:, 16:32], var[:, 0:16],
                                                 AF.Sqrt, bias=eps_sb[:, 0:1])
                            rstd = pes.tile([128, 16], F32, tag="rstd")
                            nc.vector.reciprocal(rstd[:], var[:, 16:32])
                            mu_bc = mu[:, 0:16].unsqueeze(-1) \
                                .broadcast_to((128, 16, P))
                            rs_bc = rstd[:, 0:16].unsqueeze(-1) \
                                .broadcast_to((128, 16, P))
                            w_bc = ln_w[:, wi, :].unsqueeze(1).broadcast_to(
                                (128, 16, P))
                            b_bc = ln_b[:, wi, :].unsqueeze(1).broadcast_to(
                                (128, 16, P))
                            xn = pe.tile([128, 16, P], F32, tag="xn")
                            SPL = 5
                            for eng, ts in ((nc.gpsimd, slice(0, SPL)),
                                            (nc.vector, slice(SPL, 16))):
                                n = ts.stop - ts.start
                                eng.tensor_sub(xn[:, ts], xin[:, ts],
                                               mu_bc[:, ts])
                                eng.tensor_mul(xn[:, ts], xn[:, ts],
                                               rs_bc[:, ts])
                                eng.tensor_mul(xn[:, ts], xn[:, ts],
                                               w_bc[:, ts])
                                if out_f is not None:
                                    eng.tensor_add(out_f[:, ts], xn[:, ts],
                                                   b_bc[:, ts])
                                else:
                                    eng.tensor_add(out_b[:, ts], xn[:, ts],
                                                   b_bc[:, ts])
                            if out_f is not None:
                                nc.vector.tensor_copy(out_b[:], out_f[:])

                        for rr in range(2 if ph_on('E', l) else 0):
                            xi = pe.tile([128, 16, P], BF16, tag="xi")
                            for half in range(2):
                                hs = slice(64 * half, 64 * half + 64)
                                nc.sync.dma_start(xi[hs], RSo[2 * rr + half])
                            idn_f = pe.tile([128, 16, P], F32, tag="idn_f")
                            idn_b = pe.tile([128, 16, P], BF16, tag="idn_b")
                            layernorm(xi, 0, idn_b, idn_f)
                            m1sb = [pe.tile([128, 2, 16, P], BF16,
                                            tag=f"m1sb{qq}",
                                            name=f"m1sb{qq}")
                                    for qq in range(2)]
                            for c2 in range(8):
                                cs = slice(2 * c2, 2 * c2 + 2)
                                mps = {}
                                for mb in range(2):
                                    for qq in range(2):
                                        qs = slice(64 * qq, 64 * qq + 64)
                                        mp = pep.tile([128, 512], F32,
                                                      tag=f"mp{qq}{mb}")
                                        nc.tensor.matmul(
                                            mp[:], m1w[qs, mb, :],
                                            idn_b[qs, cs, :]
                                            .rearrange("c t p -> c (t p)"),
                                            start=True, stop=True)
                                        mps[(qq, mb)] = mp
                                for mb in range(2):
                                    for qq in range(2):
                                        nc.scalar.activation(
                                            m1sb[qq][:, mb, cs, :]
                                            .rearrange("c t p -> c (t p)"),
                                            mps[(qq, mb)][:], AF.Relu,
                                            bias=m1bs[:, mb:mb + 1])
                            x2f = pe.tile([128, 16, P], F32, tag="x2f")
                            for c2 in range(8):
                                cs = slice(2 * c2, 2 * c2 + 2)
                                mp2 = pep2.tile([128, 512], F32, tag="mp2")
                                for mb in range(2):
                                    for qq in range(2):
                                        nc.tensor.matmul(
                                            mp2[64 * qq:64 * qq + 64, :],
                                            m2w[:, mb, :],
                                            m1sb[qq][:, mb, cs, :]
                                            .rearrange("c t p -> c (t p)"),
                                            start=(mb == 0), stop=(mb == 1))
                                nc.vector.scalar_tensor_tensor(
                                    x2f[:, cs, :]
                                    .rearrange("c t p -> c (t p)"),
                                    mp2[:], m2bs[:, 0:1],
                                    idn_f[:, cs, :]
                                    .rearrange("c t p -> c (t p)"),
                                    op0=ALU.add, op1=ALU.add)
                            yb = pe.tile([128, 16, P], BF16, tag="yb")
                            layernorm(x2f, 1, yb)
                            for half in range(2):
                                q = 2 * rr + half
                                hs = slice(64 * half, 64 * half + 64)
                                nc.sync.dma_start(AGi[q][:], yb[hs])
                                if l == L - 1:
                                    ybv = yb.rearrange(
                                        "c t (y x) -> c t y x", y=16)
                                    nc.sync.dma_start(
                                        StripI[:, q, 0],
                                        ybv[hs, :, 0, :].unsqueeze(1))
                                    nc.sync.dma_start(
                                        StripI[:, q, 1],
                                        ybv[hs, :, 15, :].unsqueeze(1))
                                if ph_on('R', l) and l < L - 1:
                                    nc.gpsimd.collective_compute(
                                        "AllGather", ALU.bypass,
                                        replica_groups=GROUPS,
                                        ins=[AGi[q].opt()],
                                        outs=[AGo[q].opt()])
                        if l == L - 1 and ph_on('R', l):
                            nc.gpsimd.collective_compute(
                                "AllGather", ALU.bypass,
                                replica_groups=GROUPS,
                                ins=[StripI.opt()], outs=[StripO.opt()])
                    AGi_last = AGi
                    # write gathered T back to Td (DRAM->DRAM)
                    for q in range(4 if (ph_on('R', l) and l < L - 1)
                                   else 0):
                        nc.sync.dma_start(
                            Td[:, 64 * q:64 * q + 64, :]
                            .rearrange("c (r s) p -> c r s p", r=4),
                            AGo[q].rearrange("r c s p -> c r s p"))

                # ---------- output head (rank-symmetric tile-row bands) ----
                with tc.tile_pool(name=f"H{rep}", bufs=2) as ph, \
                     tc.tile_pool(name=f"Hw{rep}", bufs=1) as phw, \
                     tc.tile_pool(name=f"Hp{rep}", bufs=2, space="PSUM") as php:
                    ow1 = phw.tile([128, 5, HID], BF16, tag="ow1")
                    ob1 = phw.tile([HID, 1], F32, tag="ob1")
                    ow2 = phw.tile([HID, 4], BF16, tag="ow2")
                    ob2 = phw.tile([4, 1], F32, tag="ob2")
                    if ph_on('H'):
                        nc.sync.dma_start(ow1[:], outw1b2_in.ap())
                        nc.sync.dma_start(ob1[:], outb1_in.ap())
                        nc.sync.dma_start(ow2[:], outw2T_in.ap())
                        nc.sync.dma_start(ob2[:], outb2_in.ap())
                    ssb = phw.tile([128, 4, 4, 2, 16, 16], BF16,
                                   tag="ssb")
                    if ph_on('H'):
                        for half in range(2):
                            hs = slice(64 * half, 64 * half + 64)
                            nc.sync.dma_start(
                                ssb[hs], StripO.rearrange(
                                    "r c q e t x -> c r q e t x"))
                    for q in range(4 if ph_on('H') else 0):
                        # own band's 16 mid rows are rank-local in AGi_last[q]
                        xi2 = ph.tile([128, 16, P], BF16, tag="xi2", bufs=2)
                        for half in range(2):
                            hs = slice(64 * half, 64 * half + 64)
                            nc.scalar.dma_start(xi2[hs], AGi_last[q][:])
                        img = ph.tile([128, 18, 16, 16], BF16, tag="img",
                                      bufs=1)
                        nc.gpsimd.memset(
                            img[:, 0, :, :].rearrange("c t x -> c (t x)"),
                            0.0)
                        nc.gpsimd.memset(
                            img[:, 17, :, :].rearrange("c t x -> c (t x)"),
                            0.0)
                        xi4 = xi2.rearrange("c t (y x) -> c t y x", y=16)
                        for y in range(16):
                            eng = nc.vector if y % 2 == 0 else nc.gpsimd
                            eng.tensor_copy(img[:, 1 + y, :, :],
                                            xi4[:, :, y, :])
                        for rr in range(4):
                            nc.vector.scalar_tensor_tensor(
                                img[:, 0, :, :], ssb[:, rr, q, 1],
                                sel_sb[:, 4 + rr:5 + rr],
                                img[:, 0, :, :], op0=ALU.mult, op1=ALU.add)
                            nc.vector.scalar_tensor_tensor(
                                img[:, 17, :, :], ssb[:, rr, q, 0],
                                sel_sb[:, 8 + rr:9 + rr],
                                img[:, 17, :, :], op0=ALU.mult, op1=ALU.add)
                        if q >= 1:
                            nc.vector.scalar_tensor_tensor(
                                img[:, 0, :, :], ssb[:, 3, q - 1, 1],
                                sel_sb[:, 12:13],
                                img[:, 0, :, :], op0=ALU.mult, op1=ALU.add)
                        if q <= 2:
                            nc.vector.scalar_tensor_tensor(
                                img[:, 17, :, :], ssb[:, 0, q + 1, 0],
                                sel_sb[:, 13:14],
                                img[:, 17, :, :], op0=ALU.mult, op1=ALU.add)
                        imgf = img.rearrange("c a t x -> c a (t x)")
                        for ch in range(8):
                            oc0 = php.tile([HID, 2, 256], F32, tag="oc0")
                            oc1 = php.tile([HID, 2, 256], F32, tag="oc1")
                            for k in range(5):
                                for ps, offl, rs in (
                                        (oc0, OFFA, slice(0, 64)),
                                        (oc1, OFFB, slice(64, 128))):
                                    dy, dx = offl[k]
                                    lix, lox, cx = _rng(dx, 256)
                                    nc.tensor.matmul(
                                        ps[:, 0:2, lox:lox + cx],
                                        ow1[rs, k, :],
                                        imgf[rs, 2 * ch + dy:2 * ch + dy + 2,
                                             lix:lix + cx],
                                        start=(k == 0), stop=(k == 4),
                                        skip_group_check=True)
                            o1 = ph.tile([HID, 2 * 256], BF16, tag="o1")
                            nc.scalar.copy(
                                o1[:], oc0.rearrange("c a x -> c (a x)"))
                            nc.vector.tensor_add(
                                o1[:], o1[:],
                                oc1.rearrange("c a x -> c (a x)"))
                            o1r = ph.tile([HID, 2 * 256], BF16, tag="o1r")
                            nc.scalar.activation(o1r[:], o1[:], AF.Relu,
                                                 bias=ob1[:, 0:1])
                            p2 = php.tile([4, 2 * 256], F32, tag="p2")
                            nc.tensor.matmul(p2[:], ow2[:], o1r[:],
                                             start=True, stop=True)
                            ysb = ph.tile([4, 2 * 256], F32, tag="ysb")
                            nc.vector.tensor_scalar_add(ysb[:], p2[:],
                                                        ob2[:, 0:1])
                            nc.sync.dma_start(
                                y_out.ap()[:, q, 2 * ch:2 * ch + 2, :]
                                .rearrange("c t x -> c (t x)"), ysb[:])
    nc.finalize()
    return nc, taps


# ======================= host side =======================

def _prep_core_inputs(core, inputs):
    r = core % 4
    b = core // 4
    f32 = np.float32
    bf = lambda a: np.ascontiguousarray(np.asarray(a, f32)).astype(NBF)
    x = np.asarray(inputs["x"], f32)
    xtc = x[b].reshape(3, 16, 16, 16, 16).transpose(0, 1, 3, 2, 4) \
        .reshape(3, T, P)
    pos = np.asarray(inputs["pos"], f32).reshape(T, HID).T.copy()
    hsl = slice(2 * r * HID, 2 * r * HID + 2 * HID)

    offa_i = [dy * 3 + dx for dy, dx in OFFA]
    offb_i = [dy * 3 + dx for dy, dx in OFFB]
    qkv2 = np.empty((L, 128, 3, 5, 128), f32)
    for l in range(L):
        for i, (nm, sc) in enumerate((("qw", 0.25), ("kw", 0.25),
                                      ("vw", 1.0))):
            w = np.asarray(inputs[nm], f32)[l, hsl] * sc  # [128,64,3,3]
            wt = w.transpose(1, 2, 3, 0).reshape(HID, 9, 2 * HID)
            for k in range(5):
                sca = 0.5 if k == 0 else 1.0
                qkv2[l, 0:64, i, k, :] = wt[:, offa_i[k], :] * sca
                qkv2[l, 64:128, i, k, :] = wt[:, offb_i[k], :] * sca
    offs_i = [dy * 3 + dx for dy, dx in OFFS]
    uwp = np.empty((L, 2 * HID, 9, HID), f32)
    for l in range(L):
        w = np.asarray(inputs["uw"], f32)[l][:, hsl]      # [64,128,3,3]
        wt = w.transpose(1, 2, 3, 0).reshape(2 * HID, 9, HID)
        uwp[l] = wt[:, offs_i, :]
    m1 = np.asarray(inputs["mlp_w1"], f32)[:, :, :, 0, 0]  # [L,256,64]
    m1T = m1.transpose(0, 2, 1).reshape(L, HID, 2, 128)    # [L,64,2,128]
    m1w2 = np.concatenate([m1T, m1T], axis=1)              # [L,128,2,128]
    m1bp = np.asarray(inputs["mlp_b1"], f32).reshape(L, 2, 2 * HID) \
        .transpose(0, 2, 1).copy()                         # [L,128,2]
    m2 = np.asarray(inputs["mlp_w2"], f32)[:, :, :, 0, 0]  # [L,64,256]
    m2Tp = m2.transpose(0, 2, 1).reshape(L, 2, 2 * HID, HID) \
        .transpose(0, 2, 1, 3).copy()                      # [L,128,2,64]
    m2b = np.asarray(inputs["mlp_b2"], f32)                # [L,64]
    m2b2 = np.concatenate([m2b, m2b], axis=1).reshape(L, 128, 1)
    lnwp = np.stack([np.asarray(inputs["ln1_w"], f32).reshape(L, HID, P),
                     np.asarray(inputs["ln2_w"], f32).reshape(L, HID, P)], 1)
    lnbp = np.stack([np.asarray(inputs["ln1_b"], f32).reshape(L, HID, P),
                     np.asarray(inputs["ln2_b"], f32).reshape(L, HID, P)], 1)
    ow1 = np.asarray(inputs["out_w1"], f32)                # [64,64,3,3]
    ow1T = ow1.transpose(1, 2, 3, 0).reshape(HID, 9, HID)
    ow1b2 = np.empty((128, 5, HID), f32)
    for k in range(5):
        sca = 0.5 if k == 0 else 1.0
        ow1b2[0:64, k, :] = ow1T[:, offa_i[k], :] * sca
        ow1b2[64:128, k, :] = ow1T[:, offb_i[k], :] * sca
    ow2 = np.asarray(inputs["out_w2"], f32)[:, :, 0, 0]    # [3,64]
    ow2T = np.zeros((HID, 4), f32)
    ow2T[:, :3] = ow2.T
    ob2 = np.zeros((4, 1), f32)
    ob2[:3, 0] = np.asarray(inputs["out_b2"], f32)
    sel = np.zeros((128, 14), f32)
    sel[:, r] = 1.0
    if r >= 1:
        sel[:, 4 + r - 1] = 1.0          # selprev
    if r <= 2:
        sel[:, 8 + r + 1] = 1.0          # selnext
    if r == 0:
        sel[:, 12] = 1.0                 # wrap from prev quarter (rank 3)
    if r == 3:
        sel[:, 13] = 1.0                 # wrap into next quarter (rank 0)
    onesblk = np.zeros((128, 128), f32)
    onesblk[0:64, 0:64] = 1.0
    onesblk[64:128, 64:128] = 1.0

    return {
        "xt": bf(xtc), "pos_in": pos.astype(np.float32),
        "semwT": bf(np.asarray(inputs["sem_w"], f32)[:, :, 0, 0].T),
        "semb": np.asarray(inputs["sem_b"], f32).reshape(HID, 1).copy(),
        "qkvw2": bf(qkv2), "uwT": bf(uwp), "m1w2": bf(m1w2), "m1b": m1bp,
        "m2T": bf(m2Tp), "m2b2": m2b2, "lnw": lnwp, "lnb": lnbp,
        "outw1b2": bf(ow1b2),
        "outb1": np.asarray(inputs["out_b1"], f32).reshape(HID, 1).copy(),
        "outw2T": bf(ow2T), "outb2": ob2,
        "ident_in": np.eye(128, dtype=NBF), "sel_in": sel,
        "onesblk_in": onesblk,
    }


def assemble_output(results):
    img = np.zeros((B, 3, 256, 256), np.float32)
    for c in range(N_CORES):
        b, r = c // 4, c % 4
        y = np.asarray(results[c]["y_out"], np.float32)  # [4,4,16,256]
        for q in range(4):
            rb = 16 * (4 * q + r)
            img[b, :, rb:rb + 16, :] = y[:3, q]
    return img


_CACHE = {}


def get_built(debug_taps=()):
    key = tuple(sorted(debug_taps))
    if key not in _CACHE:
        t0 = time.time()
        nc, taps = build(debug_taps)
        _CACHE[key] = (nc, taps)
        print(f"[kernel] build {time.time() - t0:.1f}s", file=sys.stderr)
    return _CACHE[key]


def run_cores(inputs, debug_taps=()):
    from concourse import bass2jax
    nc, taps = get_built(debug_taps)
    in_maps = [_prep_core_inputs(c, inputs) for c in range(N_CORES)]
    t0 = time.time()
    results = bass2jax.run_bass_via_pjrt(nc, in_maps, n_cores=N_CORES)
    print(f"[kernel] run {time.time() - t0:.1f}s", file=sys.stderr)
    return results


def kernel(**inputs):
    results = run_cores(inputs)
    return assemble_output(results)
